# revision 1
# baseline (speedup 1.0000x reference)
"""Trainium2 Bass kernel for nn_ATK_SPA_87351044866230 (sparse_attention).

Sharding: 8 cores = 2 batches x 4 h-chunks of 64 rows (1-row halo for the
3x3 depthwise conv). Params replicated.

v2 pipeline per core (all convs folded into PE matmuls):
  x tiles -> PE transpose -> x1cm (f32, padded cols) / x2buf (bf16)
  q~,k~,v~ = dwconv3x3(qkv(x1)) computed directly as 9 accumulating f32r
    matmuls per 512-px chunk with host-folded weights diag(dw_t) @ Wg
  gate: g1 = relu(W1@[x1;x2]), g2 rows collected in one PSUM tile [32,512]
  Gram: q~,k~ pairs transposed (f16) -> Gram accumulation in PSUM
  tiny all-8 AllReduce for gate mean; 2-group AllReduce [gram|sq_q|sq_k]
  attn: normalize, rank via 16 compares, mask, softmax
  P1eff = blockdiag(attn) @ (projT1 * asum)  (one matmul)
  out[px,:] = v~_tile^T @ P1eff + x2_tile^T @ projT2  (2 matmuls/128 px)
"""
import numpy as np

B, H, W, DIM = 2, 256, 256, 256
PDIM, HEADS, CH = 128, 8, 16
N_CORES = 8
ROWS = 64            # output rows per core
HALO_ROWS = ROWS + 2
WP = W + 2           # padded row length
BLK = 4              # row blocks per core
BR = 16              # output rows per block
BIR = BR + 2         # input rows per block
NPB = BR * W         # out pixels per block (4096)
TAPS = [(dy, dx) for dy in range(3) for dx in range(3)]


def _build_program():
    import contextlib
    import concourse.bass as bass
    import concourse.bacc as bacc
    import concourse.mybir as mybir
    from concourse import masks
    from concourse.tile import TileContext

    dt = mybir.dt
    f32, f16, bf16 = dt.float32, dt.float16, dt.bfloat16
    f32r = dt.float32r
    AF = mybir.ActivationFunctionType
    ALU = mybir.AluOpType

    nc = bacc.Bacc(trn_type="TRN2", target_bir_lowering=False,
                   num_devices=N_CORES)

    # ---- I/O ----
    xs = nc.dram_tensor("xs", [HALO_ROWS * W, DIM], f32r, kind="ExternalInput").ap()
    wfold_d = nc.dram_tensor("wfold", [PDIM, 27 * PDIM], f32r, kind="ExternalInput").ap()
    g1wTa_d = nc.dram_tensor("g1wTa", [PDIM, PDIM], f32r, kind="ExternalInput").ap()
    g1wTb_d = nc.dram_tensor("g1wTb", [PDIM, PDIM], f32, kind="ExternalInput").ap()
    g1b_d = nc.dram_tensor("g1b", [PDIM, 1], f32, kind="ExternalInput").ap()
    g2rep_d = nc.dram_tensor("g2rep", [PDIM, 32], f32, kind="ExternalInput").ap()
    g2b_d = nc.dram_tensor("g2b", [32, 1], f32, kind="ExternalInput").ap()
    projT1_d = nc.dram_tensor("projT1", [PDIM, DIM], f32r, kind="ExternalInput").ap()
    projT2_d = nc.dram_tensor("projT2", [PDIM, DIM], f32, kind="ExternalInput").ap()
    tempb_d = nc.dram_tensor("tempb", [PDIM, 1], f32, kind="ExternalInput").ap()
    bmask_d = nc.dram_tensor("bmask", [PDIM, PDIM], f32, kind="ExternalInput").ap()
    ys = nc.dram_tensor("ys", [ROWS * W, DIM], f32, kind="ExternalOutput").ap()

    with TileContext(nc) as tc:
        ctx = contextlib.ExitStack()
        with ctx:
            pw = ctx.enter_context(tc.tile_pool(name="pw", bufs=1))
            pbig = ctx.enter_context(tc.tile_pool(name="pbig", bufs=1))
            pxs = ctx.enter_context(tc.tile_pool(name="pxs", bufs=2))
            pg1 = ctx.enter_context(tc.tile_pool(name="pg1", bufs=2))
            pev = ctx.enter_context(tc.tile_pool(name="pev", bufs=3))
            pout = ctx.enter_context(tc.tile_pool(name="pout", bufs=8))
            psm = ctx.enter_context(tc.tile_pool(name="psm", bufs=1))
            pdram = ctx.enter_context(tc.tile_pool(name="pdram", bufs=2, space="DRAM"))
            # PSUM pools
            ptx = ctx.enter_context(tc.tile_pool(name="ptx", bufs=2, space="PSUM"))
            pmm = ctx.enter_context(tc.tile_pool(name="pmm", bufs=3, space="PSUM"))
            ptq = ctx.enter_context(tc.tile_pool(name="ptq", bufs=2, space="PSUM"))
            pgram = ctx.enter_context(tc.tile_pool(name="pgram", bufs=1, space="PSUM"))

            # ---- prefetch first block's x tiles before weights ----
            xst0s = []
            for half in range(2):
                xst = pxs.tile([128, BIR * DIM], f32r, tag="xst", name="xst0")
                xst_r = xst[:].rearrange("p (t c) -> p t c", c=DIM)
                base = half * BIR * 128
                for ck in range(6):
                    nc.sync.dma_start(
                        xst_r[:, ck * 3:(ck + 1) * 3, :],
                        xs[base + ck * 3 * 128:base + (ck + 1) * 3 * 128, :]
                        .rearrange("(t p) c -> p t c", p=128))
                xst0s.append(xst)

            # ---- load weights/constants into SBUF ----
            w_fold = pw.tile([PDIM, 27 * PDIM], f32r)
            nc.sync.dma_start(w_fold[:], wfold_d[:])
            w_g1Ta = pw.tile([PDIM, PDIM], f32r)
            nc.sync.dma_start(w_g1Ta[:], g1wTa_d[:])
            w_g1Tf = pw.tile([PDIM, PDIM], f32)
            nc.sync.dma_start(w_g1Tf[:], g1wTb_d[:])
            w_g1b = pw.tile([PDIM, 1], f32)
            nc.sync.dma_start(w_g1b[:], g1b_d[:])
            w_g2rep_f = pw.tile([PDIM, 32], f32)
            nc.sync.dma_start(w_g2rep_f[:], g2rep_d[:])
            w_g2b = pw.tile([32, 1], f32)
            nc.sync.dma_start(w_g2b[:], g2b_d[:])
            w_pT1 = pw.tile([PDIM, DIM], f32r)
            nc.sync.dma_start(w_pT1[:], projT1_d[:])
            w_pT2f = pw.tile([PDIM, DIM], f32)
            nc.sync.dma_start(w_pT2f[:], projT2_d[:])
            w_temp = pw.tile([PDIM, 1], f32)
            nc.sync.dma_start(w_temp[:], tempb_d[:])
            w_bmask = pw.tile([PDIM, PDIM], f32)
            nc.sync.dma_start(w_bmask[:], bmask_d[:])

            w_projT2 = pw.tile([PDIM, DIM], bf16)
            nc.vector.tensor_copy(w_projT2[:], w_pT2f[:])
            w_g1Tb = pw.tile([PDIM, PDIM], bf16)
            nc.vector.tensor_copy(w_g1Tb[:], w_g1Tf[:])
            w_g2T = pw.tile([PDIM, 1], bf16)
            nc.vector.tensor_copy(w_g2T[:], w_g2rep_f[:, 0:1])
            w_g2b1 = pw.tile([1, 1], f32)
            nc.vector.tensor_copy(w_g2b1[:], w_g2b[0:1, :])

            ident_f32 = pw.tile([128, 128], f32)
            masks.make_identity(nc, ident_f32[:])
            ident_f16 = pw.tile([128, 128], f16)
            masks.make_identity(nc, ident_f16[:])
            ident_f32r = pw.tile([128, 128], f32r)
            nc.vector.tensor_copy(ident_f32r[:], ident_f32[:])
            w_zero = pw.tile([128, 1], f32)
            nc.gpsimd.memset(w_zero[:], 0.0)
            w_negone = pw.tile([1, 1], f32)
            nc.gpsimd.memset(w_negone[:], -1.0)
            ones_row = pw.tile([1, 128], f32)
            nc.gpsimd.memset(ones_row[:], 1.0)

            # ---- persistent big buffers ----
            x1cm = pbig.tile([PDIM, BIR * WP], f32r)    # padded channel-major x1
            nc.gpsimd.memset(x1cm[:].bitcast(f32), 0.0)
            x2buf = pbig.tile([PDIM, ROWS * W], bf16)
            vtil = pbig.tile([PDIM, ROWS * W], bf16)    # v~ = dwconv(v)
            dwout = pbig.tile([PDIM, 2 * NPB], f16)     # q~,k~ per blk
            sqst = pbig.tile([PDIM, 2], f32)            # [sq_q, sq_k]
            sqscr = pbig.tile([PDIM, NPB], f16)         # square scratch
            sqscr2 = pbig.tile([PDIM, NPB], f16)
            sqcols = pbig.tile([PDIM, 8], f32)          # q: 0..3, k: 4..7
            gst = pbig.tile([PDIM, PDIM], f32)
            garbuf = pbig.tile([PDIM, 130], f32)
            gslocal = pbig.tile([1, 4], f32)
            nc.gpsimd.memset(gslocal[:], 0.0)
            gcols = pbig.tile([1, 32], f32)
            sgscr = pbig.tile([1, 512], f32)
            gsglob = pbig.tile([1, 4], f32)
            bd = pbig.tile([128, 128], f32r)            # blockdiag(attn)
            p1eff = pbig.tile([PDIM, DIM], bf16)

            gram_ps = pgram.tile([128, 128], f32, tag="gram")

            x1_r = x1cm[:].rearrange("p (r w) -> p r w", w=WP)
            n_gram_mm = (NPB // 128) * BLK  # 128 total
            gram_i = 0

            # ================= PHASE 1 =================
            for blk in range(BLK):
                # --- load + transpose x (2 staged half-blocks, 9 rows each) ---
                if blk == 0:
                    xsts = xst0s
                else:
                    xsts = []
                    for half in range(2):
                        xst = pxs.tile([128, BIR * DIM], f32r, tag="xst")
                        base = blk * BR * W + half * BIR * 128
                        xst_r = xst[:].rearrange("p (t c) -> p t c", c=DIM)
                        for ck in range(3):  # split for earlier readiness
                            nc.sync.dma_start(
                                xst_r[:, ck * 6:(ck + 1) * 6, :],
                                xs[base + ck * 6 * 128:base + (ck + 1) * 6 * 128, :]
                                .rearrange("(t p) c -> p t c", p=128))
                        xsts.append(xst)
                px1 = px2 = None
                for t in range(2 * BIR):  # 36 pixel-tiles of 128
                    half, ti = t // BIR, t % BIR
                    src = xsts[half][:].rearrange("p (t c) -> p t c", c=DIM)
                    q1 = t % 4
                    if q1 == 0:
                        px1 = ptx.tile([128, 512], f32r, tag="ptx")
                    nc.tensor.transpose(px1[:, q1 * 128:(q1 + 1) * 128],
                                        src[:, ti, 0:128], ident_f32r[:])
                    if q1 == 3:
                        r = (t // 4) * 2
                        nc.vector.tensor_copy(
                            x1_r[:, r:r + 2, 1:1 + W],
                            px1[:].rearrange("p (r w) -> p r w", w=W))
                    if 2 <= t < 2 * BIR - 2:
                        q2 = (t - 2) % 4
                        if q2 == 0:
                            px2 = ptx.tile([128, 512], f32r, tag="ptx")
                        nc.tensor.transpose(px2[:, q2 * 128:(q2 + 1) * 128],
                                            src[:, ti, 128:256], ident_f32r[:])
                        if q2 == 3:
                            gcol = blk * NPB + (t - 5) * 128
                            nc.scalar.copy(x2buf[:, gcol:gcol + 512], px2[:])

                # --- gate ---
                for chk in range(BR // 2):
                    g1p = pmm.tile([128, 512], f32, tag="pmm")
                    rhs1 = x1_r[:, 1 + chk * 2:3 + chk * 2, 1:1 + W]
                    nc.tensor.matmul(g1p[:], w_g1Ta[:], rhs1,
                                     start=True, stop=False)
                    rhs2 = x2buf[:, blk * NPB + chk * 512:blk * NPB + (chk + 1) * 512]
                    nc.tensor.matmul(g1p[:], w_g1Tb[:], rhs2,
                                     start=False, stop=True)
                    g1s = pg1.tile([128, 512], bf16, tag="g1s")
                    nc.scalar.activation(g1s[:], g1p[:], AF.Relu, bias=w_g1b[:, 0:1])
                    row = blk * (BR // 2) + chk
                    nc.tensor.matmul(g1p[0:1, :], w_g2T[:], g1s[:],
                                     start=True, stop=True)
                    nc.scalar.activation(sgscr[:], g1p[0:1, :], AF.Sigmoid,
                                         bias=w_g2b1[0:1, 0:1],
                                         accum_out=gcols[0:1, row:row + 1])

                if blk == BLK - 1:
                    # gate mean: tiny all-8 AllReduce (hidden under compute)
                    nc.vector.reduce_sum(gslocal[0:1, 0:1], gcols[0:1, :],
                                         axis=mybir.AxisListType.X)
                    ing = pdram.tile([1, 4], f32, tag="ing")
                    outg = pdram.tile([1, 4], f32, tag="outg",
                                      addr_space="Shared")
                    nc.sync.dma_start(ing[:], gslocal[:])
                    nc.gpsimd.collective_compute(
                        "AllReduce", mybir.AluOpType.add,
                        replica_groups=[list(range(N_CORES))],
                        ins=[ing[:].opt()], outs=[outg[:].opt()])
                    nc.gpsimd.dma_start(gsglob[:], outg[:])

                # --- folded dwconv+qkv, 9 f32r matmuls per chunk ---
                def folded(g, dst, dst_off):
                    for pr in range(4):  # pairs of 512-px chunks
                        pps = [pmm.tile([128, 512], f32, tag="pmm",
                                        name=f"pp{s}")
                               for s in range(2)]
                        for tidx, (dy, dx) in enumerate(TAPS):
                            lhsT = w_fold[:, (g * 9 + tidx) * 128:
                                          (g * 9 + tidx + 1) * 128]
                            for s in range(2):
                                r0 = (pr * 2 + s) * 2
                                rhs = x1_r[:, r0 + dy:r0 + dy + 2, dx:dx + W]
                                nc.tensor.matmul(pps[s][:], lhsT, rhs,
                                                 start=(tidx == 0),
                                                 stop=(tidx == 8))
                        for s in range(2):
                            col = dst_off + (pr * 2 + s) * 512
                            nc.scalar.copy(dst[:, col:col + 512], pps[s][:])

                folded(0, dwout, 0)          # q~
                folded(1, dwout, NPB)        # k~

                # --- q~,k~ transpose pairs + Gram accumulation (f16) ---
                for t2 in range(NPB // 256):  # 2 pixel-tiles per eviction
                    tp = ptq.tile([128, 512], f16, tag="ptq")
                    for k in range(2):
                        tt = t2 * 2 + k
                        nc.tensor.transpose(
                            tp[:, k * 128:(k + 1) * 128],
                            dwout[:, tt * 128:(tt + 1) * 128], ident_f16[:])
                        nc.tensor.transpose(
                            tp[:, 256 + k * 128:256 + (k + 1) * 128],
                            dwout[:, NPB + tt * 128:NPB + (tt + 1) * 128],
                            ident_f16[:])
                    ev = pev.tile([128, 512], f16, tag="ev")
                    if t2 % 2 == 0:
                        nc.vector.tensor_copy(ev[:], tp[:])
                    else:
                        nc.scalar.copy(ev[:], tp[:])
                    for k in range(2):
                        nc.tensor.matmul(
                            gram_ps[:], ev[:, k * 128:(k + 1) * 128],
                            ev[:, 256 + k * 128:256 + (k + 1) * 128],
                            start=(gram_i == 0),
                            stop=(gram_i == n_gram_mm - 1))
                        gram_i += 1

                # sums of squares (after gram: keeps copy queues clear)
                nc.scalar.activation(
                    sqscr[:], dwout[:, 0:NPB], AF.Square, bias=w_zero[:, 0:1],
                    accum_out=sqcols[:, blk:blk + 1])
                nc.vector.tensor_mul(sqscr2[:], dwout[:, NPB:2 * NPB],
                                     dwout[:, NPB:2 * NPB])
                nc.vector.reduce_sum(sqcols[:, 4 + blk:5 + blk], sqscr2[:],
                                     axis=mybir.AxisListType.X)

                if blk == BLK - 1:
                    # main 2-group AllReduce: [gram | sq_q | sq_k]
                    nc.vector.reduce_sum(sqst[:, 0:1], sqcols[:, 0:4],
                                         axis=mybir.AxisListType.X)
                    nc.vector.reduce_sum(sqst[:, 1:2], sqcols[:, 4:8],
                                         axis=mybir.AxisListType.X)
                    nc.scalar.copy(gst[:], gram_ps[:])
                    inb = pdram.tile([PDIM, 130], f32, tag="inb")
                    outb = pdram.tile([PDIM, 130], f32, tag="outb")
                    nc.sync.dma_start(inb[:, 0:128], gst[:])
                    nc.sync.dma_start(inb[:, 128:130], sqst[:])
                    nc.gpsimd.collective_compute(
                        "AllReduce", mybir.AluOpType.add,
                        replica_groups=[[0, 1, 2, 3], [4, 5, 6, 7]],
                        ins=[inb[:].opt()], outs=[outb[:].opt()])
                    nc.gpsimd.dma_start(garbuf[:], outb[:])

                folded(2, vtil, blk * NPB)   # v~ (hides the AllReduce)

            tc.strict_bb_all_engine_barrier()

            # ================= attn (tiny per-head CxC) =================
            # threshold = 16*mean(g) - 1
            thr = psm.tile([1, 1], f32)
            nc.scalar.activation(thr[:], gsglob[0:1, 0:1], AF.Identity,
                                 scale=float(CH) / float(B * H * W),
                                 bias=w_negone[0:1, 0:1])
            thrB_ps = pmm.tile([128, 512], f32, tag="pmm")
            nc.tensor.matmul(thrB_ps[:, 0:1], ones_row[:], thr[:],
                             start=True, stop=True)
            thr_b = psm.tile([128, 1], f32)
            nc.scalar.copy(thr_b[:], thrB_ps[:, 0:1])
            # norms: n = 1/max(sqrt(sq), 1e-12); fold temperature into q side
            nq = psm.tile([128, 1], f32)
            nc.scalar.activation(nq[:], garbuf[:, 128:129], AF.Sqrt,
                                 bias=w_zero[:, 0:1])
            nc.vector.tensor_scalar_max(nq[:], nq[:], 1e-12)
            nc.vector.reciprocal(nq[:], nq[:])
            nc.vector.tensor_mul(nq[:], nq[:], w_temp[:])
            nk = psm.tile([128, 1], f32)
            nc.scalar.activation(nk[:], garbuf[:, 129:130], AF.Sqrt,
                                 bias=w_zero[:, 0:1])
            nc.vector.tensor_scalar_max(nk[:], nk[:], 1e-12)
            nc.vector.reciprocal(nk[:], nk[:])

            # scale G rows by nq, cols by nk; extract diag blocks -> at16
            nkT_ps = pmm.tile([128, 512], f32, tag="pmm")
            nc.tensor.transpose(nkT_ps[0:1, 0:128], nk[:], ident_f32[:])
            nkTs = psm.tile([1, 128], f32)
            nc.scalar.copy(nkTs[:], nkT_ps[0:1, 0:128])
            nkB_ps = pmm.tile([128, 512], f32, tag="pmm")
            nc.tensor.matmul(nkB_ps[:, 0:128], ones_row[:], nkTs[:],
                             start=True, stop=True)
            nkB = psm.tile([128, 128], f32)
            nc.scalar.copy(nkB[:], nkB_ps[:, 0:128])
            nc.vector.tensor_scalar_mul(garbuf[:, 0:128], garbuf[:, 0:128],
                                        nq[:, 0:1])
            nc.vector.tensor_mul(garbuf[:, 0:128], garbuf[:, 0:128], nkB[:])
            gm = psm.tile([128, 128], f32)
            nc.vector.tensor_mul(gm[:], garbuf[:, 0:128], w_bmask[:])
            at16 = psm.tile([128, 16], f32)
            nc.vector.reduce_sum(
                at16[:], gm[:].rearrange("p (d j) -> p j d", j=16),
                axis=mybir.AxisListType.X)

            # ranks: rk[p,j] = #{j' : at16[p,j'] > at16[p,j]}
            rk = psm.tile([128, 16], f32)
            nc.vector.tensor_scalar(rk[:], at16[:], at16[:, 0:1], None,
                                    op0=ALU.is_lt)
            for j in range(1, 16):
                nc.vector.scalar_tensor_tensor(
                    rk[:], at16[:], at16[:, j:j + 1], rk[:],
                    op0=ALU.is_lt, op1=ALU.add)
            mask = psm.tile([128, 16], f32)
            nc.vector.tensor_scalar(mask[:], rk[:], thr_b[:, 0:1], None,
                                    op0=ALU.is_lt)
            # masked softmax (global row max is always kept, k >= 1)
            mx = psm.tile([128, 1], f32)
            nc.vector.reduce_max(mx[:], at16[:], axis=mybir.AxisListType.X)
            mxn = psm.tile([128, 1], f32)
            nc.vector.tensor_scalar_mul(mxn[:], mx[:], -1.0)
            e16 = psm.tile([128, 16], f32)
            nc.scalar.activation(e16[:], at16[:], AF.Exp, bias=mxn[:, 0:1])
            nc.vector.tensor_mul(e16[:], e16[:], mask[:])
            ssum = psm.tile([128, 1], f32)
            nc.vector.reduce_sum(ssum[:], e16[:], axis=mybir.AxisListType.X)
            nc.vector.reciprocal(ssum[:], ssum[:])
            nc.vector.tensor_scalar_mul(e16[:], e16[:], ssum[:, 0:1])

            # P1eff = blockdiag(attn) @ projT1s  (asum folded host-side)
            nc.vector.tensor_mul(
                bd[:].rearrange("p (d j) -> p d j", j=16),
                e16[:].unsqueeze(1).broadcast_to([128, HEADS, 16]),
                w_bmask[:].rearrange("p (d j) -> p d j", j=16))
            p1p = pmm.tile([128, 512], f32, tag="pmm")
            nc.tensor.matmul(p1p[:, 0:DIM], bd[:], w_pT1[:],
                             start=True, stop=True)
            nc.scalar.copy(p1eff[:], p1p[:, 0:DIM])

            # ================= PHASE 2: proj + output =================
            for i in range(ROWS * W // 256):  # 128 pairs of 128-px tiles
                gp = i * 256
                pp = pmm.tile([128, 512], f32, tag="pmm")
                for tile in range(2):
                    o0 = tile * 256
                    nc.tensor.matmul(pp[:, o0:o0 + 256],
                                     vtil[:, gp + tile * 128:gp + tile * 128 + 128],
                                     p1eff[:], start=True, stop=False)
                    nc.tensor.matmul(pp[:, o0:o0 + 256],
                                     x2buf[:, gp + tile * 128:gp + tile * 128 + 128],
                                     w_projT2[:], start=False, stop=True)
                ot = pout.tile([128, 512], f32, tag="ot")
                if i % 2 == 0:
                    nc.scalar.copy(ot[:], pp[:])
                else:
                    nc.vector.tensor_copy(ot[:], pp[:])
                nc.sync.dma_start(
                    ys[gp:gp + 256, :].rearrange("(t p) c -> p t c", p=128),
                    ot[:].rearrange("p (t c) -> p t c", c=DIM))

    nc.finalize()
    return nc


_CACHED = {}


def _get_results(in_maps):
    from concourse.bass_utils import run_bass_kernel_spmd
    if "nc" not in _CACHED:
        _CACHED["nc"] = _build_program()
    nc = _CACHED["nc"]
    return run_bass_kernel_spmd(nc, in_maps, list(range(N_CORES)))


def _prep_inputs(x, qkv_w, dw_w, proj_w, g1_w, g1_b, g2_w, g2_b,
                 temperature, attn1, attn2, attn3, attn4):
    x = np.asarray(x, np.float32)
    wT = np.ascontiguousarray(np.asarray(qkv_w, np.float32).T)         # [128, 384]
    dwf = np.asarray(dw_w, np.float32).reshape(3 * PDIM, 9)            # [384, 9]
    # wfold[c, g, t, o] = wT[c, g*128+o] * dwf[g*128+o, t]
    wfold = (wT.reshape(PDIM, 3, 1, PDIM)
             * dwf.reshape(3, PDIM, 9).transpose(0, 2, 1)[None])       # [128,3,9,128]
    wfold = np.ascontiguousarray(wfold.reshape(PDIM, 27 * PDIM))
    g1wT = np.asarray(g1_w, np.float32).T                              # [256, 128]
    g1wTa = np.ascontiguousarray(g1wT[0:PDIM])
    g1wTb = np.ascontiguousarray(g1wT[PDIM:2 * PDIM])
    g1b = np.asarray(g1_b, np.float32).reshape(PDIM, 1)
    g2rep = np.ascontiguousarray(
        np.repeat(np.asarray(g2_w, np.float32).reshape(PDIM, 1), 32, axis=1))
    g2b = np.full((32, 1), float(np.asarray(g2_b).reshape(-1)[0]), np.float32)
    projT = np.ascontiguousarray(np.asarray(proj_w, np.float32).T)     # [256, 256]
    tempb = np.repeat(np.asarray(temperature, np.float32).reshape(HEADS), CH)
    tempb = np.ascontiguousarray(tempb.reshape(PDIM, 1))
    bmask = np.zeros((PDIM, PDIM), np.float32)
    for h in range(HEADS):
        bmask[h * CH:(h + 1) * CH, h * CH:(h + 1) * CH] = 1.0
    asum = float(sum(np.asarray(a, np.float32).reshape(-1)[0]
                     for a in (attn1, attn2, attn3, attn4)))
    projT1 = np.ascontiguousarray(projT[0:PDIM] * asum)
    projT2 = np.ascontiguousarray(projT[PDIM:2 * PDIM])

    in_maps = []
    for cid in range(N_CORES):
        b = cid // 4
        r0 = (cid % 4) * ROWS
        xsh = np.zeros((HALO_ROWS, W, DIM), np.float32)
        lo, hi = r0 - 1, r0 + ROWS + 1
        slo, shi = max(lo, 0), min(hi, H)
        xsh[slo - lo:shi - lo] = x[b, slo:shi]
        in_maps.append(dict(
            xs=np.ascontiguousarray(xsh.reshape(HALO_ROWS * W, DIM)),
            wfold=wfold, g1wTa=g1wTa, g1wTb=g1wTb, g1b=g1b,
            g2rep=g2rep, g2b=g2b,
            projT1=projT1, projT2=projT2, tempb=tempb, bmask=bmask,
        ))
    return in_maps


def kernel(x, qkv_w, dw_w, proj_w, g1_w, g1_b, g2_w, g2_b,
           temperature, attn1, attn2, attn3, attn4):
    in_maps = _prep_inputs(x, qkv_w, dw_w, proj_w, g1_w, g1_b, g2_w, g2_b,
                           temperature, attn1, attn2, attn3, attn4)
    res = _get_results(in_maps)
    out = np.zeros((B, H, W, DIM), np.float32)
    for cid in range(N_CORES):
        b = cid // 4
        r0 = (cid % 4) * ROWS
        out[b, r0:r0 + ROWS] = np.asarray(
            res.results[cid]["ys"], np.float32).reshape(ROWS, W, DIM)
    return out



# revision 75
# speedup vs baseline: 1.3743x; 1.3743x over previous
"""Trainium2 Bass kernel for nn_ATK_SPA_87351044866230 (sparse_attention).

Sharding: 8 cores = 2 batches x 4 h-chunks of 64 rows (1-row halo for the
3x3 depthwise conv). Params replicated.

v3 pipeline per core:
  x tiles (f32) -> f16 convert (Act/Pool) -> PE transpose (f16, 1.0 c/row)
  q~,k~ = dwconv3x3(qkv(x1)) as 9 accumulating f16 matmuls per chunk
    (host-folded weights diag(dw_t) @ Wg), taps-outer loop over 4-chunk halves
  v~ = same but fp8e4 DoubleRow: 4 tap-pairs (2 k-tiles each, 0.5 c/row)
    + 1 plain fp8 tap; x1 fp8 copy made on Pool per block
  gate: g1 = relu(W1@[x1;x2]) f16, g2 rows -> per-block accum columns
  Gram: q~,k~ f16 transposes -> f16 Gram accumulation in PSUM (all blocks)
  ONE all-8 AllGather of [128,19] (per-head gram diag + sq_q/sq_k + gate),
    batch-local shard sums via host-passed 0/1 select mask; collective is
    hidden under the deferred v~ folds
  attn: normalize, rank via 16 compares, mask, softmax  (all [128,16])
  P1eff = blockdiag(attn) @ (projT1 * asum)
  phase2 (transposed out): outT[o,px] = P1effT@v~ + projT2T@x2, stationary
    proj weights, moving v~/x2 chunks; bf16 stores, host un-transposes
"""
import numpy as np

B, H, W, DIM = 2, 256, 256, 256
PDIM, HEADS, CH = 128, 8, 16
N_CORES = 8
ROWS = 64            # output rows per core
HALO_ROWS = ROWS + 2
WP = W + 2           # padded row length
BLK = 4              # row blocks per core
BR = 16              # output rows per block
BIR = BR + 2         # input rows per block
NPB = BR * W         # out pixels per block (4096)
TAPS = [(dy, dx) for dy in range(3) for dx in range(3)]
# v-conv fp8 DoubleRow tap pairs. HW constraint: the k-tile window offset
# delta must be EVEN in elements (odd fp8 deltas crash the exec unit), so
# pair dx 0<->2 within a row (delta 2) and (0,1)<->(1,1) (delta WP=258);
# tap 7 runs as a plain fp8 matmul.
VPAIRS = [(0, 2), (3, 5), (6, 8), (1, 4)]
VLONE = 7
DEBUG = False


def _build_program():
    import contextlib
    import concourse.bass as bass
    import concourse.bacc as bacc
    import concourse.mybir as mybir
    from concourse import masks
    from concourse.tile import TileContext

    dt = mybir.dt
    f32, f16, bf16, f8 = dt.float32, dt.float16, dt.bfloat16, dt.float8e4
    f32r = dt.float32r
    AF = mybir.ActivationFunctionType
    ALU = mybir.AluOpType
    DR = mybir.MatmulPerfMode.DoubleRow

    nc = bacc.Bacc(trn_type="TRN2", target_bir_lowering=False,
                   num_devices=N_CORES)

    # ---- I/O ----
    xs = nc.dram_tensor("xs", [HALO_ROWS * W, DIM], f32r, kind="ExternalInput").ap()
    wfold_d = nc.dram_tensor("wfold", [PDIM, 27 * PDIM], f16, kind="ExternalInput").ap()
    g1wTa_d = nc.dram_tensor("g1wTa", [PDIM, PDIM], f16, kind="ExternalInput").ap()
    g1wTb_d = nc.dram_tensor("g1wTb", [PDIM, PDIM], f16, kind="ExternalInput").ap()
    g1b_d = nc.dram_tensor("g1b", [PDIM, 1], f32, kind="ExternalInput").ap()
    g2T_d = nc.dram_tensor("g2T", [PDIM, 1], f16, kind="ExternalInput").ap()
    g2b_d = nc.dram_tensor("g2b", [1, 1], f32, kind="ExternalInput").ap()
    projT1_d = nc.dram_tensor("projT1", [PDIM, DIM], f16, kind="ExternalInput").ap()
    projT2_d = nc.dram_tensor("projT2", [PDIM, DIM], f16, kind="ExternalInput").ap()
    tempb_d = nc.dram_tensor("tempb", [PDIM, 1], f32, kind="ExternalInput").ap()
    bmask_d = nc.dram_tensor("bmask", [PDIM, PDIM], f32, kind="ExternalInput").ap()
    bmask16_d = nc.dram_tensor("bmask16", [PDIM, PDIM], f16, kind="ExternalInput").ap()
    maskJ_d = nc.dram_tensor("maskJ", [PDIM, CH], f16, kind="ExternalInput").ap()
    selB_d = nc.dram_tensor("selB", [PDIM, N_CORES], f32, kind="ExternalInput").ap()
    ys = nc.dram_tensor("ys", [DIM, ROWS * W], bf16, kind="ExternalOutput").ap()
    dbg = {}
    if DEBUG:
        for name, shape, dty in [
                ("d_x2buf", [PDIM, ROWS * W], f16),
                ("d_vtil", [PDIM, ROWS * W], f16),
                ("d_dwout3", [PDIM, 2 * NPB], f16),
                ("d_garbuf", [PDIM, N_CORES * 19], f32),
                ("d_arbuf", [PDIM, 19], f32),
                ("d_agg", [PDIM, 19], f32),
                ("d_at16", [PDIM, CH], f32),
                ("d_rk", [PDIM, CH], f32),
                ("d_e16", [PDIM, CH], f32),
                ("d_p1eff", [PDIM, DIM], f16),
                ("d_x8a0", [PDIM, BIR * WP], f8)]:
            dbg[name] = nc.dram_tensor(name, shape, dty,
                                       kind="ExternalOutput").ap()

    AGW = 19                      # allgather payload cols per core
    NQ = 4                        # quarters per block (9 px-tiles each)
    QT = 9                        # px-tiles per quarter

    with TileContext(nc) as tc:
        ctx = contextlib.ExitStack()
        with ctx:
            pw = ctx.enter_context(tc.tile_pool(name="pw", bufs=1))
            pbig = ctx.enter_context(tc.tile_pool(name="pbig", bufs=1))
            pxq = ctx.enter_context(tc.tile_pool(name="pxq", bufs=3))
            pxq16 = ctx.enter_context(tc.tile_pool(name="pxq16", bufs=3))
            pg1 = ctx.enter_context(tc.tile_pool(name="pg1", bufs=2))
            pev = ctx.enter_context(tc.tile_pool(name="pev", bufs=2))
            psq = ctx.enter_context(tc.tile_pool(name="psq", bufs=2))
            pout = ctx.enter_context(tc.tile_pool(name="pout", bufs=6))
            psm = ctx.enter_context(tc.tile_pool(name="psm", bufs=1))
            pdram = ctx.enter_context(tc.tile_pool(name="pdram", bufs=2, space="DRAM"))
            # PSUM pools (8 banks of 2KB: phase1 3+2+2+1, phase2 7+1)
            pgram = ctx.enter_context(tc.tile_pool(name="pgram", bufs=1, space="PSUM"))
            ph1 = contextlib.ExitStack()
            pfold = ph1.enter_context(tc.tile_pool(name="pfold", bufs=3, space="PSUM"))
            ptx = ph1.enter_context(tc.tile_pool(name="ptx", bufs=2, space="PSUM"))
            ptq = ph1.enter_context(tc.tile_pool(name="ptq", bufs=2, space="PSUM"))

            # ---- prefetch first block's x quarters before weights ----
            def load_quarter(blk, q):
                xq = pxq.tile([128, QT * DIM], f32r, tag="xq")
                xq_r = xq[:].rearrange("p (t c) -> p t c", c=DIM)
                base = blk * BR * W + q * QT * 128
                for ck in range(3):
                    nc.sync.dma_start(
                        xq_r[:, ck * 3:(ck + 1) * 3, :],
                        xs[base + ck * 3 * 128:base + (ck + 1) * 3 * 128, :]
                        .rearrange("(t p) c -> p t c", p=128))
                return xq

            # interleave block-0 prefetch with weight loads ordered by first
            # use (conv weights before the later quarters, misc last)
            xq0s = [load_quarter(0, q) for q in range(2)]

            w_fold = pw.tile([PDIM, 27 * PDIM], f16)
            nc.sync.dma_start(w_fold[:], wfold_d[:])
            w_g1Ta = pw.tile([PDIM, PDIM], f16)
            nc.sync.dma_start(w_g1Ta[:], g1wTa_d[:])
            w_g1Tb = pw.tile([PDIM, PDIM], f16)
            nc.sync.dma_start(w_g1Tb[:], g1wTb_d[:])
            w_g1b = pw.tile([PDIM, 1], f32)
            nc.sync.dma_start(w_g1b[:], g1b_d[:])
            w_g2T = pw.tile([PDIM, 1], f16)
            nc.sync.dma_start(w_g2T[:], g2T_d[:])
            w_g2b = pw.tile([1, 1], f32)
            nc.sync.dma_start(w_g2b[:], g2b_d[:])

            xq0s += [load_quarter(0, q) for q in range(2, NQ)]

            w_pT1 = pw.tile([PDIM, DIM], f16)
            nc.sync.dma_start(w_pT1[:], projT1_d[:])
            w_pT2 = pw.tile([PDIM, DIM], f16)
            nc.sync.dma_start(w_pT2[:], projT2_d[:])
            w_temp = pw.tile([PDIM, 1], f32)
            nc.sync.dma_start(w_temp[:], tempb_d[:])
            w_bmask = pw.tile([PDIM, PDIM], f32)
            nc.sync.dma_start(w_bmask[:], bmask_d[:])
            w_bmask16 = pw.tile([PDIM, PDIM], f16)
            nc.sync.dma_start(w_bmask16[:], bmask16_d[:])
            w_maskJ = pw.tile([PDIM, CH], f16)
            nc.sync.dma_start(w_maskJ[:], maskJ_d[:])
            w_selB = pw.tile([PDIM, N_CORES], f32)
            nc.sync.dma_start(w_selB[:], selB_d[:])

            ident_f16 = pw.tile([128, 128], f16)
            masks.make_identity(nc, ident_f16[:])
            w_zero = pw.tile([128, 1], f32)
            nc.gpsimd.memset(w_zero[:], 0.0)
            w_negone = pw.tile([1, 1], f32)
            nc.gpsimd.memset(w_negone[:], -1.0)
            ones_row = pw.tile([1, 128], f32)
            nc.gpsimd.memset(ones_row[:], 1.0)

            # fp8 copy of v-group folded weights (taps 18..26)
            w_fold8v = pw.tile([PDIM, 9 * PDIM], f8)
            nc.vector.tensor_copy(w_fold8v[:], w_fold[:, 18 * PDIM:27 * PDIM])

            # ---- persistent big buffers ----
            x1cms = [pbig.tile([PDIM, BIR * WP], f16, name=f"x1cm{i}")
                     for i in range(2)]
            for t in x1cms:
                # zero only the pad columns (0, 257); per-block evictions
                # overwrite cols 1..256. Tiny DVE writes instead of a 3.9us
                # Pool memset that stalls the first block's evictions.
                t_r = t[:].rearrange("p (r w) -> p r w", w=WP)
                for pc in (0, WP - 1):
                    nc.vector.tensor_copy(
                        t_r[:, :, pc:pc + 1],
                        w_zero[:].unsqueeze(1).broadcast_to([PDIM, BIR, 1]))
            x8as = [pbig.tile([PDIM, BIR * WP], f8, name=f"x8a{i}")
                    for i in range(BLK)]
            x2buf = pbig.tile([PDIM, ROWS * W], f16)
            vtil = pbig.tile([PDIM, ROWS * W], f16)
            dwouts = [pbig.tile([PDIM, 2 * NPB], f16, name=f"dwout{i}")
                      for i in range(2)]
            sqcols = pbig.tile([PDIM, 64], f32)    # q: 0..31, k: 32..63
            gcols = pbig.tile([1, 32], f32)
            sgscr = pbig.tile([1, 512], f16)
            garm = pbig.tile([PDIM, PDIM], f32)
            arbuf = pbig.tile([PDIM, AGW], f32)
            garbuf = pbig.tile([PDIM, N_CORES * AGW], f32)

            gram_ps = pgram.tile([128, 128], f32, tag="gram")
            n_gram_mm = (NPB // 128) * BLK  # 128 total
            gram_i = 0

            # ================= PHASE 1 =================
            for blk in range(BLK):
                x1cm = x1cms[blk % 2]
                dwout = dwouts[blk % 2]
                x1_r = x1cm[:].rearrange("p (r w) -> p r w", w=WP)

                # --- load + convert + transpose x (4 quarters, 9 tiles) ---
                xqs = xq0s if blk == 0 else [load_quarter(blk, q)
                                             for q in range(NQ)]
                xq16s = []
                for q in range(NQ):
                    xq16 = pxq16.tile([128, QT * DIM], f16, tag="xq16")
                    if blk == 0:
                        # chunked conversion so transposes start early; DVE
                        # (Act is busy with act-table loads at startup)
                        for ck in range(3):
                            c0, c1 = ck * 3 * DIM, (ck + 1) * 3 * DIM
                            nc.vector.tensor_copy(xq16[:, c0:c1],
                                                  xqs[q][:, c0:c1])
                    else:
                        # Pool cast, chunked so transposes don't wait on the
                        # whole-quarter conversion latency
                        for ck in range(3):
                            c0, c1 = ck * 3 * DIM, (ck + 1) * 3 * DIM
                            nc.gpsimd.tensor_copy(xq16[:, c0:c1],
                                                  xqs[q][:, c0:c1])
                    xq16s.append(xq16)

                # px1 in [0:512] of a paired [128,1024] PSUM tile (bank-sized),
                # px2 (offset by 2 tiles) in [512:1024] of the same tile
                ptile = px2t = None
                for t in range(2 * BIR):  # 36 pixel-tiles of 128
                    src = xq16s[t // QT][:].rearrange("p (t c) -> p t c", c=DIM)
                    ti = t % QT
                    q1 = t % 4
                    if q1 == 0:
                        ptile = ptx.tile([128, 1024], f16, tag="ptx")
                    nc.tensor.transpose(ptile[:, q1 * 128:(q1 + 1) * 128],
                                        src[:, ti, 0:128], ident_f16[:])
                    if q1 == 3:
                        r = (t // 4) * 2
                        nc.vector.tensor_copy(
                            x1_r[:, r:r + 2, 1:1 + W],
                            ptile[:, 0:512].rearrange("p (r w) -> p r w", w=W))
                    if 2 <= t < 2 * BIR - 2:
                        q2 = (t - 2) % 4
                        if q2 == 0:
                            px2t = ptile
                        nc.tensor.transpose(
                            px2t[:, 512 + q2 * 128:512 + (q2 + 1) * 128],
                            src[:, ti, 128:256], ident_f16[:])
                        if q2 == 3:
                            gcol = blk * NPB + (t - 5) * 128
                            nc.vector.tensor_copy(x2buf[:, gcol:gcol + 512],
                                                  px2t[:, 512:1024])

                # fp8 copy of x1 for the v-conv (includes zero pads)
                nc.scalar.copy(x8as[blk][:], x1cm[:])

                # --- gate ---
                for chk in range(BR // 2):
                    g1p = pfold.tile([128, 512], f32, tag="pfold")
                    rhs1 = x1_r[:, 1 + chk * 2:3 + chk * 2, 1:1 + W]
                    nc.tensor.matmul(g1p[:], w_g1Ta[:], rhs1,
                                     start=True, stop=False)
                    rhs2 = x2buf[:, blk * NPB + chk * 512:blk * NPB + (chk + 1) * 512]
                    nc.tensor.matmul(g1p[:], w_g1Tb[:], rhs2,
                                     start=False, stop=True)
                    g1s = pg1.tile([128, 512], f16, tag="g1s")
                    nc.scalar.activation(g1s[:], g1p[:], AF.Relu, bias=w_g1b[:, 0:1])
                    row = blk * (BR // 2) + chk
                    nc.tensor.matmul(g1p[0:1, :], w_g2T[:], g1s[:],
                                     start=True, stop=True)
                    nc.scalar.activation(sgscr[:], g1p[0:1, :], AF.Sigmoid,
                                         bias=w_g2b[0:1, 0:1],
                                         accum_out=gcols[0:1, row:row + 1])

                # --- folded dwconv+qkv for q,k: taps-outer over 2-chunk pairs;
                # squares and qkT/gram interleaved per-hp so block-3's gram
                # (the collective's dependency) completes right after folds ---
                for g in range(2):
                    for hp in range(4):
                        pps = [pfold.tile([128, 512], f32, tag="pfold",
                                          name=f"pp{s}") for s in range(2)]
                        for tidx in range(9):
                            dy, dx = TAPS[tidx]
                            lhsT = w_fold[:, (g * 9 + tidx) * 128:
                                          (g * 9 + tidx + 1) * 128]
                            for s in range(2):
                                r0 = (hp * 2 + s) * 2
                                rhs = x1_r[:, r0 + dy:r0 + dy + 2, dx:dx + W]
                                nc.tensor.matmul(pps[s][:], lhsT, rhs,
                                                 start=(tidx == 0),
                                                 stop=(tidx == 8))
                        for s in range(2):
                            col = g * NPB + (hp * 2 + s) * 512
                            if s % 2 == 0:
                                nc.vector.tensor_copy(dwout[:, col:col + 512],
                                                      pps[s][:])
                            else:
                                nc.scalar.copy(dwout[:, col:col + 512], pps[s][:])
                        for s in range(2):
                            chk = hp * 2 + s
                            col = blk * 8 + chk
                            if g == 0:
                                if s == 0:
                                    sqa = psq.tile([128, 512], f16, tag="sqa")
                                    nc.scalar.activation(
                                        sqa[:], dwout[:, chk * 512:(chk + 1) * 512],
                                        AF.Square, bias=w_zero[:, 0:1],
                                        accum_out=sqcols[:, col:col + 1])
                                else:
                                    sqa = psq.tile([128, 512], f16, tag="sqa")
                                    nc.vector.tensor_mul(
                                        sqa[:], dwout[:, chk * 512:(chk + 1) * 512],
                                        dwout[:, chk * 512:(chk + 1) * 512])
                                    nc.vector.reduce_sum(
                                        sqcols[:, col:col + 1], sqa[:],
                                        axis=mybir.AxisListType.X)
                            else:
                                sqb = psq.tile([128, 512], f16, tag="sqa")
                                nc.vector.tensor_mul(
                                    sqb[:],
                                    dwout[:, NPB + chk * 512:NPB + (chk + 1) * 512],
                                    dwout[:, NPB + chk * 512:NPB + (chk + 1) * 512])
                                nc.vector.reduce_sum(
                                    sqcols[:, 32 + col:33 + col], sqb[:],
                                    axis=mybir.AxisListType.X)
                        if g == 1:
                            # q,k transposes + gram for this hp's px tiles
                            tq = None
                            for t2 in range(4 * hp, 4 * hp + 4):
                                if t2 % 2 == 0:
                                    tq = ptq.tile([128, 1024], f16, tag="ptq")
                                tb = (t2 % 2) * 512
                                for k in range(2):
                                    tt = t2 * 2 + k
                                    nc.tensor.transpose(
                                        tq[:, tb + k * 128:tb + (k + 1) * 128],
                                        dwout[:, tt * 128:(tt + 1) * 128],
                                        ident_f16[:])
                                    nc.tensor.transpose(
                                        tq[:, tb + 256 + k * 128:
                                           tb + 256 + (k + 1) * 128],
                                        dwout[:, NPB + tt * 128:
                                              NPB + (tt + 1) * 128],
                                        ident_f16[:])
                                ev = pev.tile([128, 512], f16, tag="ev")
                                nc.vector.tensor_copy(ev[:], tq[:, tb:tb + 512])
                                for k in range(2):
                                    nc.tensor.matmul(
                                        gram_ps[:], ev[:, k * 128:(k + 1) * 128],
                                        ev[:, 256 + k * 128:256 + (k + 1) * 128],
                                        start=(gram_i == 0),
                                        stop=(gram_i == n_gram_mm - 1))
                                    gram_i += 1

            # ================= AllGather staging =================
            # compact gram to per-head diag blocks [128,16]
            nc.vector.tensor_mul(garm[:], gram_ps[:], w_bmask[:])
            nc.vector.reduce_sum(
                arbuf[:, 0:16], garm[:].rearrange("p (d j) -> p j d", j=16),
                axis=mybir.AxisListType.X)
            nc.vector.reduce_sum(arbuf[:, 16:17], sqcols[:, 0:32],
                                 axis=mybir.AxisListType.X)
            nc.vector.reduce_sum(arbuf[:, 17:18], sqcols[:, 32:64],
                                 axis=mybir.AxisListType.X)
            nc.vector.tensor_copy(arbuf[:, 18:19], w_zero[:])
            nc.vector.reduce_sum(arbuf[0:1, 18:19], gcols[0:1, :],
                                 axis=mybir.AxisListType.X)
            inb = pdram.tile([PDIM, AGW], f32, tag="inb")
            # AllGather concatenates raveled per-rank buffers: [s, p, j]
            outag = pdram.tile([N_CORES * PDIM, AGW], f32, tag="outag",
                               addr_space="Shared")
            nc.sync.dma_start(inb[:], arbuf[:])
            nc.gpsimd.collective_compute(
                "AllGather", mybir.AluOpType.bypass,
                replica_groups=[list(range(N_CORES))],
                ins=[inb[:].opt()], outs=[outag[:].opt()])
            # gpsimd queue: a DMA holds its issuing queue's SEQ while waiting
            # for the collective; Pool is idle here, sync is not
            nc.gpsimd.dma_start(
                garbuf[:].rearrange("p (s j) -> p s j", j=AGW),
                outag[:].rearrange("(s p) j -> p s j", p=PDIM))

            # ================= v~ folds (hide the collective) =================
            # blocks 0-1 on the phase-1 pool (overlapping its drain barrier),
            # blocks 2-3 + attn + phase2 on a deep 7-bank pool
            pp2 = None
            wf8_r = w_fold8v[:].rearrange("p (t m) -> p t m", m=128)
            for blk in range(BLK):
                if blk == 2:
                    ph1.close()
                    pp2 = ctx.enter_context(
                        tc.tile_pool(name="pp2", bufs=7, space="PSUM"))
                vpool, vtag = (pfold, "pfold") if blk < 2 else (pp2, "vps")
                x8_r = x8as[blk][:].rearrange("p (r w) -> p r w", w=WP)
                for hp in range(4):
                    pps = [vpool.tile([128, 512], f32, tag=vtag,
                                      name=f"vp{s}") for s in range(2)]
                    # plain fp8 lone tap first: starts (zeroes) the full tile
                    dyL, dxL = TAPS[VLONE]
                    lhsTL = wf8_r[:, VLONE, :]
                    for s in range(2):
                        r0 = (hp * 2 + s) * 2
                        rhs = x8_r[:, r0 + dyL:r0 + dyL + 2, dxL:dxL + W]
                        nc.tensor.matmul(pps[s][:], lhsTL, rhs,
                                         start=True, stop=False)
                    for pi, (ta, tb) in enumerate(VPAIRS):
                        dyA, dxA = TAPS[ta]
                        dyB, dxB = TAPS[tb]
                        delta = (dyB - dyA) * WP + (dxB - dxA)
                        lhsT = wf8_r[:, ta:ta + 1, :].copy()
                        lhsT.ap[1] = [(tb - ta) * 128, 2]  # [128, 2, 128]
                        for s in range(2):
                            r0 = (hp * 2 + s) * 2
                            for r in range(2):
                                w0 = x8_r[:, r0 + dyA + r:r0 + dyA + r + 1,
                                          dxA:dxA + W]
                                wpair = w0.copy()
                                wpair.ap[1] = [delta, 2]
                                nc.tensor.matmul(
                                    pps[s][:, r * 256:(r + 1) * 256],
                                    lhsT, wpair, start=False,
                                    stop=(pi == 3 and r == 1), perf_mode=DR,
                                    skip_group_check=True)
                    for s in range(2):
                        col = blk * NPB + (hp * 2 + s) * 512
                        if s % 2 == 0:
                            nc.vector.tensor_copy(vtil[:, col:col + 512],
                                                  pps[s][:])
                        else:
                            nc.scalar.copy(vtil[:, col:col + 512], pps[s][:])

            # pre-issue the x2 half of phase-2 segment 0: it doesn't need the
            # attn result and bridges part of the collective latency
            pre2 = {}
            for oh in range(2):
                for c in range(2):
                    pt = pp2.tile([128, 512], f32, tag="vps", name="pre")
                    lx = w_pT2[:, oh * 128:(oh + 1) * 128]
                    gp = c * 512
                    nc.tensor.matmul(pt[:], lx, x2buf[:, gp:gp + 512],
                                     start=True, stop=False)
                    pre2[(oh, c)] = pt

            # ================= attn (tiny per-head CxC) =================
            # batch-local shard sums: agg[p, j] = sum_s sel[s] * shard_s[p, j]
            gar_r = garbuf[:].rearrange("p (s j) -> p s j", j=AGW)
            agsel = psm.tile([PDIM, N_CORES * AGW], f32)
            ag_r = agsel[:].rearrange("p (s j) -> p s j", j=AGW)
            nc.vector.tensor_mul(
                ag_r[:, :, :], gar_r[:, :, :],
                w_selB[:].unsqueeze(2).broadcast_to([PDIM, N_CORES, AGW]))
            agg = psm.tile([PDIM, AGW], f32)
            nc.vector.reduce_sum(
                agg[:], agsel[:].rearrange("p (s j) -> p j s", j=AGW),
                axis=mybir.AxisListType.X)
            # gate global sum over all 8 shards
            gsum = psm.tile([1, 1], f32)
            nc.vector.reduce_sum(
                gsum[:], gar_r[0:1, :, 18:19].rearrange("p s j -> p (j s)"),
                axis=mybir.AxisListType.X)
            # threshold = 16*mean(g) - 1, broadcast to partitions
            thr = psm.tile([1, 1], f32)
            nc.scalar.activation(thr[:], gsum[0:1, 0:1], AF.Identity,
                                 scale=float(CH) / float(B * H * W),
                                 bias=w_negone[0:1, 0:1])
            thrB_ps = pp2.tile([128, 512], f32, tag="vps")
            nc.tensor.matmul(thrB_ps[:, 0:1], ones_row[:], thr[:],
                             start=True, stop=True)
            thr_b = psm.tile([128, 1], f32)
            nc.scalar.copy(thr_b[:], thrB_ps[:, 0:1])
            # norms: nq = temp/max(sqrt(sq_q),eps), nk = 1/max(sqrt(sq_k),eps)
            nq = psm.tile([128, 1], f32)
            nc.scalar.activation(nq[:], agg[:, 16:17], AF.Sqrt,
                                 bias=w_zero[:, 0:1])
            nc.vector.tensor_scalar_max(nq[:], nq[:], 1e-12)
            nc.vector.reciprocal(nq[:], nq[:])
            nc.vector.tensor_mul(nq[:], nq[:], w_temp[:])
            nk = psm.tile([128, 1], f32)
            nc.scalar.activation(nk[:], agg[:, 17:18], AF.Sqrt,
                                 bias=w_zero[:, 0:1])
            nc.vector.tensor_scalar_max(nk[:], nk[:], 1e-12)
            nc.vector.reciprocal(nk[:], nk[:])
            # nkB16[c, j] = nk[head(c)*16 + j] via bmask matmul trick
            nkS = psm.tile([128, CH], f16)
            nc.vector.tensor_scalar_mul(nkS[:], w_maskJ[:], nk[:, 0:1])
            nkB_ps = pp2.tile([128, 512], f32, tag="vps")
            nc.tensor.matmul(nkB_ps[:, 0:CH], w_bmask16[:], nkS[:],
                             start=True, stop=True)
            at16 = psm.tile([128, CH], f32)
            nc.vector.scalar_tensor_tensor(at16[:], agg[:, 0:16], nq[:, 0:1],
                                           nkB_ps[:, 0:CH],
                                           op0=ALU.mult, op1=ALU.mult)

            # ranks: rk[p,j] = #{j' : at16[p,j'] > at16[p,j]} via one broadcast
            # compare [128,16,16] + reduce over j'
            cmp = psm.tile([128, CH * CH], f32)
            nc.vector.tensor_tensor(
                cmp[:].rearrange("p (a b) -> p a b", b=CH),
                at16[:].unsqueeze(2).broadcast_to([128, CH, CH]),
                at16[:].unsqueeze(1).broadcast_to([128, CH, CH]),
                op=ALU.is_lt)
            rk = psm.tile([128, CH], f32)
            nc.vector.reduce_sum(rk[:],
                                 cmp[:].rearrange("p (a b) -> p a b", b=CH),
                                 axis=mybir.AxisListType.X)
            mask = psm.tile([128, CH], f32)
            nc.vector.tensor_scalar(mask[:], rk[:], thr_b[:, 0:1], None,
                                    op0=ALU.is_lt)
            # masked softmax (global row max is always kept, k >= 1)
            mx = psm.tile([128, 1], f32)
            nc.vector.reduce_max(mx[:], at16[:], axis=mybir.AxisListType.X)
            mxn = psm.tile([128, 1], f32)
            nc.vector.tensor_scalar_mul(mxn[:], mx[:], -1.0)
            e16 = psm.tile([128, CH], f32)
            nc.scalar.activation(e16[:], at16[:], AF.Exp, bias=mxn[:, 0:1])
            nc.vector.tensor_mul(e16[:], e16[:], mask[:])
            ssum = psm.tile([128, 1], f32)
            nc.vector.reduce_sum(ssum[:], e16[:], axis=mybir.AxisListType.X)
            nc.vector.reciprocal(ssum[:], ssum[:])
            nc.vector.tensor_scalar_mul(e16[:], e16[:], ssum[:, 0:1])

            # P1eff = blockdiag(attn) @ projT1s  (asum folded host-side)
            bd = psm.tile([128, 128], f16)
            nc.vector.tensor_mul(
                bd[:].rearrange("p (d j) -> p d j", j=16),
                e16[:].unsqueeze(1).broadcast_to([128, HEADS, 16]),
                w_bmask16[:].rearrange("p (d j) -> p d j", j=16))
            p1p = pp2.tile([128, 512], f32, tag="vps")
            nc.tensor.matmul(p1p[:, 0:DIM], bd[:], w_pT1[:],
                             start=True, stop=True)
            p1eff = psm.tile([PDIM, DIM], f16)
            nc.scalar.copy(p1eff[:], p1p[:, 0:DIM])

            if DEBUG:
                nc.sync.dma_start(dbg["d_x2buf"][:], x2buf[:])
                nc.sync.dma_start(dbg["d_vtil"][:], vtil[:])
                nc.sync.dma_start(dbg["d_dwout3"][:], dwouts[1][:])
                nc.sync.dma_start(dbg["d_garbuf"][:], garbuf[:])
                nc.sync.dma_start(dbg["d_arbuf"][:], arbuf[:])
                nc.sync.dma_start(dbg["d_agg"][:], agg[:])
                nc.sync.dma_start(dbg["d_at16"][:], at16[:])
                nc.sync.dma_start(dbg["d_rk"][:], rk[:])
                nc.sync.dma_start(dbg["d_e16"][:], e16[:])
                nc.sync.dma_start(dbg["d_p1eff"][:], p1eff[:])
                nc.sync.dma_start(dbg["d_x8a0"][:], x8as[0][:])

            # ================= PHASE 2: transposed proj + output ============
            # outT[o, px] = P1eff[:,o]^T @ v~ + projT2[:,o]^T @ x2
            for seg in range(ROWS * W // 1024):  # 16 segments of 1024 px
                for oh in range(2):
                    if seg == 0:
                        pps = [pre2[(oh, c)] for c in range(2)]
                    else:
                        pps = [pp2.tile([128, 512], f32, tag="vps",
                                        name=f"o{c}") for c in range(2)]
                        # x2 half first: independent of the attn result
                        lx = w_pT2[:, oh * 128:(oh + 1) * 128]
                        for c in range(2):
                            gp = seg * 1024 + c * 512
                            nc.tensor.matmul(pps[c][:], lx,
                                             x2buf[:, gp:gp + 512],
                                             start=True, stop=False)
                    lv = p1eff[:, oh * 128:(oh + 1) * 128]
                    for c in range(2):
                        gp = seg * 1024 + c * 512
                        nc.tensor.matmul(pps[c][:], lv, vtil[:, gp:gp + 512],
                                         start=False, stop=True)
                    ot = pout.tile([128, 1024], bf16, tag="ot")
                    nc.vector.tensor_copy(ot[:, 0:512], pps[0][:])
                    nc.scalar.copy(ot[:, 512:1024], pps[1][:])
                    dst = ys[oh * 128:(oh + 1) * 128,
                             seg * 1024:(seg + 1) * 1024]
                    if (seg * 2 + oh) % 2 == 1:
                        nc.sync.dma_start(dst, ot[:])
                    else:
                        nc.gpsimd.dma_start(dst, ot[:])

    nc.finalize()
    return nc


_CACHED = {}


def _get_results(in_maps):
    from concourse.bass_utils import run_bass_kernel_spmd
    if "nc" not in _CACHED:
        _CACHED["nc"] = _build_program()
    nc = _CACHED["nc"]
    return run_bass_kernel_spmd(nc, in_maps, list(range(N_CORES)))


def _prep_inputs(x, qkv_w, dw_w, proj_w, g1_w, g1_b, g2_w, g2_b,
                 temperature, attn1, attn2, attn3, attn4):
    x = np.asarray(x, np.float32)
    wT = np.ascontiguousarray(np.asarray(qkv_w, np.float32).T)         # [128, 384]
    dwf = np.asarray(dw_w, np.float32).reshape(3 * PDIM, 9)            # [384, 9]
    # wfold[c, g, t, o] = wT[c, g*128+o] * dwf[g*128+o, t]
    wfold = (wT.reshape(PDIM, 3, 1, PDIM)
             * dwf.reshape(3, PDIM, 9).transpose(0, 2, 1)[None])       # [128,3,9,128]
    wfold = np.ascontiguousarray(wfold.reshape(PDIM, 27 * PDIM)).astype(np.float16)
    g1wT = np.asarray(g1_w, np.float32).T                              # [256, 128]
    g1wTa = np.ascontiguousarray(g1wT[0:PDIM]).astype(np.float16)
    g1wTb = np.ascontiguousarray(g1wT[PDIM:2 * PDIM]).astype(np.float16)
    g1b = np.asarray(g1_b, np.float32).reshape(PDIM, 1)
    g2T = np.asarray(g2_w, np.float32).reshape(PDIM, 1).astype(np.float16)
    g2b = np.asarray(g2_b, np.float32).reshape(1, 1)
    projT = np.ascontiguousarray(np.asarray(proj_w, np.float32).T)     # [256, 256]
    tempb = np.repeat(np.asarray(temperature, np.float32).reshape(HEADS), CH)
    tempb = np.ascontiguousarray(tempb.reshape(PDIM, 1))
    bmask = np.zeros((PDIM, PDIM), np.float32)
    for h in range(HEADS):
        bmask[h * CH:(h + 1) * CH, h * CH:(h + 1) * CH] = 1.0
    bmask16 = bmask.astype(np.float16)
    maskJ = np.zeros((PDIM, CH), np.float16)
    for d in range(PDIM):
        maskJ[d, d % CH] = 1.0
    asum = float(sum(np.asarray(a, np.float32).reshape(-1)[0]
                     for a in (attn1, attn2, attn3, attn4)))
    projT1 = np.ascontiguousarray(projT[0:PDIM] * asum).astype(np.float16)
    projT2 = np.ascontiguousarray(projT[PDIM:2 * PDIM]).astype(np.float16)

    in_maps = []
    for cid in range(N_CORES):
        b = cid // 4
        r0 = (cid % 4) * ROWS
        xsh = np.zeros((HALO_ROWS, W, DIM), np.float32)
        lo, hi = r0 - 1, r0 + ROWS + 1
        slo, shi = max(lo, 0), min(hi, H)
        xsh[slo - lo:shi - lo] = x[b, slo:shi]
        selB = np.zeros((PDIM, N_CORES), np.float32)
        selB[:, 4 * b:4 * b + 4] = 1.0
        in_maps.append(dict(
            xs=np.ascontiguousarray(xsh.reshape(HALO_ROWS * W, DIM)),
            wfold=wfold, g1wTa=g1wTa, g1wTb=g1wTb, g1b=g1b,
            g2T=g2T, g2b=g2b,
            projT1=projT1, projT2=projT2, tempb=tempb,
            bmask=bmask, bmask16=bmask16, maskJ=maskJ, selB=selB,
        ))
    return in_maps


def kernel(x, qkv_w, dw_w, proj_w, g1_w, g1_b, g2_w, g2_b,
           temperature, attn1, attn2, attn3, attn4):
    in_maps = _prep_inputs(x, qkv_w, dw_w, proj_w, g1_w, g1_b, g2_w, g2_b,
                           temperature, attn1, attn2, attn3, attn4)
    res = _get_results(in_maps)
    out = np.zeros((B, H, W, DIM), np.float32)
    for cid in range(N_CORES):
        b = cid // 4
        r0 = (cid % 4) * ROWS
        yt = np.asarray(res.results[cid]["ys"], np.float32)  # [DIM, ROWS*W]
        out[b, r0:r0 + ROWS] = yt.reshape(DIM, ROWS, W).transpose(1, 2, 0)
    return out


# revision 76
# speedup vs baseline: 1.3776x; 1.0024x over previous
"""Trainium2 Bass kernel for nn_ATK_SPA_87351044866230 (sparse_attention).

Sharding: 8 cores = 2 batches x 4 h-chunks of 64 rows (1-row halo for the
3x3 depthwise conv). Params replicated.

v3 pipeline per core:
  x tiles (f32) -> f16 convert (Act/Pool) -> PE transpose (f16, 1.0 c/row)
  q~,k~ = dwconv3x3(qkv(x1)) as 9 accumulating f16 matmuls per chunk
    (host-folded weights diag(dw_t) @ Wg), taps-outer loop over 4-chunk halves
  v~ = same but fp8e4 DoubleRow: 4 tap-pairs (2 k-tiles each, 0.5 c/row)
    + 1 plain fp8 tap; x1 fp8 copy made on Pool per block
  gate: g1 = relu(W1@[x1;x2]) f16, g2 rows -> per-block accum columns
  Gram: q~,k~ f16 transposes -> f16 Gram accumulation in PSUM (all blocks)
  ONE all-8 AllGather of [128,19] (per-head gram diag + sq_q/sq_k + gate),
    batch-local shard sums via host-passed 0/1 select mask; collective is
    hidden under the deferred v~ folds
  attn: normalize, rank via 16 compares, mask, softmax  (all [128,16])
  P1eff = blockdiag(attn) @ (projT1 * asum)
  phase2 (transposed out): outT[o,px] = P1effT@v~ + projT2T@x2, stationary
    proj weights, moving v~/x2 chunks; bf16 stores, host un-transposes
"""
import numpy as np

B, H, W, DIM = 2, 256, 256, 256
PDIM, HEADS, CH = 128, 8, 16
N_CORES = 8
ROWS = 64            # output rows per core
HALO_ROWS = ROWS + 2
WP = W + 2           # padded row length
BLK = 4              # row blocks per core
BR = 16              # output rows per block
BIR = BR + 2         # input rows per block
NPB = BR * W         # out pixels per block (4096)
TAPS = [(dy, dx) for dy in range(3) for dx in range(3)]
# v-conv fp8 DoubleRow tap pairs. HW constraint: the k-tile window offset
# delta must be EVEN in elements (odd fp8 deltas crash the exec unit), so
# pair dx 0<->2 within a row (delta 2) and (0,1)<->(1,1) (delta WP=258);
# tap 7 runs as a plain fp8 matmul.
VPAIRS = [(0, 2), (3, 5), (6, 8), (1, 4)]
VLONE = 7
DEBUG = False


def _build_program():
    import contextlib
    import concourse.bass as bass
    import concourse.bacc as bacc
    import concourse.mybir as mybir
    from concourse import masks
    from concourse.tile import TileContext

    dt = mybir.dt
    f32, f16, bf16, f8 = dt.float32, dt.float16, dt.bfloat16, dt.float8e4
    f32r = dt.float32r
    AF = mybir.ActivationFunctionType
    ALU = mybir.AluOpType
    DR = mybir.MatmulPerfMode.DoubleRow

    nc = bacc.Bacc(trn_type="TRN2", target_bir_lowering=False,
                   num_devices=N_CORES)

    # ---- I/O ----
    xs = nc.dram_tensor("xs", [HALO_ROWS * W, DIM], f32r, kind="ExternalInput").ap()
    wfold_d = nc.dram_tensor("wfold", [PDIM, 27 * PDIM], f16, kind="ExternalInput").ap()
    g1wTa_d = nc.dram_tensor("g1wTa", [PDIM, PDIM], f16, kind="ExternalInput").ap()
    g1wTb_d = nc.dram_tensor("g1wTb", [PDIM, PDIM], f16, kind="ExternalInput").ap()
    g1b_d = nc.dram_tensor("g1b", [PDIM, 1], f32, kind="ExternalInput").ap()
    g2T_d = nc.dram_tensor("g2T", [PDIM, 1], f16, kind="ExternalInput").ap()
    g2b_d = nc.dram_tensor("g2b", [1, 1], f32, kind="ExternalInput").ap()
    projT1_d = nc.dram_tensor("projT1", [PDIM, DIM], f16, kind="ExternalInput").ap()
    projT2_d = nc.dram_tensor("projT2", [PDIM, DIM], f16, kind="ExternalInput").ap()
    tempb_d = nc.dram_tensor("tempb", [PDIM, 1], f32, kind="ExternalInput").ap()
    bmask_d = nc.dram_tensor("bmask", [PDIM, PDIM], f32, kind="ExternalInput").ap()
    bmask16_d = nc.dram_tensor("bmask16", [PDIM, PDIM], f16, kind="ExternalInput").ap()
    maskJ_d = nc.dram_tensor("maskJ", [PDIM, CH], f16, kind="ExternalInput").ap()
    selB_d = nc.dram_tensor("selB", [PDIM, N_CORES], f32, kind="ExternalInput").ap()
    ys = nc.dram_tensor("ys", [DIM, ROWS * W], bf16, kind="ExternalOutput").ap()
    dbg = {}
    if DEBUG:
        for name, shape, dty in [
                ("d_x2buf", [PDIM, ROWS * W], f16),
                ("d_vtil", [PDIM, ROWS * W], f16),
                ("d_dwout3", [PDIM, 2 * NPB], f16),
                ("d_garbuf", [PDIM, N_CORES * 19], f32),
                ("d_arbuf", [PDIM, 19], f32),
                ("d_agg", [PDIM, 19], f32),
                ("d_at16", [PDIM, CH], f32),
                ("d_rk", [PDIM, CH], f32),
                ("d_e16", [PDIM, CH], f32),
                ("d_p1eff", [PDIM, DIM], f16),
                ("d_x8a0", [PDIM, BIR * WP], f8)]:
            dbg[name] = nc.dram_tensor(name, shape, dty,
                                       kind="ExternalOutput").ap()

    AGW = 19                      # allgather payload cols per core
    NQ = 4                        # quarters per block (9 px-tiles each)
    QT = 9                        # px-tiles per quarter

    with TileContext(nc) as tc:
        ctx = contextlib.ExitStack()
        with ctx:
            pw = ctx.enter_context(tc.tile_pool(name="pw", bufs=1))
            pbig = ctx.enter_context(tc.tile_pool(name="pbig", bufs=1))
            pxq = ctx.enter_context(tc.tile_pool(name="pxq", bufs=3))
            pxq16 = ctx.enter_context(tc.tile_pool(name="pxq16", bufs=3))
            pg1 = ctx.enter_context(tc.tile_pool(name="pg1", bufs=2))
            pev = ctx.enter_context(tc.tile_pool(name="pev", bufs=2))
            psq = ctx.enter_context(tc.tile_pool(name="psq", bufs=2))
            pout = ctx.enter_context(tc.tile_pool(name="pout", bufs=6))
            psm = ctx.enter_context(tc.tile_pool(name="psm", bufs=1))
            pdram = ctx.enter_context(tc.tile_pool(name="pdram", bufs=2, space="DRAM"))
            # PSUM pools (8 banks of 2KB: phase1 3+2+2+1, phase2 7+1)
            pgram = ctx.enter_context(tc.tile_pool(name="pgram", bufs=1, space="PSUM"))
            ph1 = contextlib.ExitStack()
            pfold = ph1.enter_context(tc.tile_pool(name="pfold", bufs=3, space="PSUM"))
            ptx = ph1.enter_context(tc.tile_pool(name="ptx", bufs=2, space="PSUM"))
            ptq = ph1.enter_context(tc.tile_pool(name="ptq", bufs=2, space="PSUM"))

            # ---- prefetch first block's x quarters before weights ----
            def load_quarter(blk, q):
                xq = pxq.tile([128, QT * DIM], f32r, tag="xq")
                xq_r = xq[:].rearrange("p (t c) -> p t c", c=DIM)
                base = blk * BR * W + q * QT * 128
                for ck in range(3):
                    nc.sync.dma_start(
                        xq_r[:, ck * 3:(ck + 1) * 3, :],
                        xs[base + ck * 3 * 128:base + (ck + 1) * 3 * 128, :]
                        .rearrange("(t p) c -> p t c", p=128))
                return xq

            # interleave block-0 prefetch with weight loads ordered by first
            # use (conv weights before the later quarters, misc last)
            xq0s = [load_quarter(0, q) for q in range(2)]

            w_fold = pw.tile([PDIM, 27 * PDIM], f16)
            nc.sync.dma_start(w_fold[:], wfold_d[:])
            w_g1Ta = pw.tile([PDIM, PDIM], f16)
            nc.sync.dma_start(w_g1Ta[:], g1wTa_d[:])
            w_g1Tb = pw.tile([PDIM, PDIM], f16)
            nc.sync.dma_start(w_g1Tb[:], g1wTb_d[:])
            w_g1b = pw.tile([PDIM, 1], f32)
            nc.sync.dma_start(w_g1b[:], g1b_d[:])
            w_g2T = pw.tile([PDIM, 1], f16)
            nc.sync.dma_start(w_g2T[:], g2T_d[:])
            w_g2b = pw.tile([1, 1], f32)
            nc.sync.dma_start(w_g2b[:], g2b_d[:])

            xq0s += [load_quarter(0, q) for q in range(2, NQ)]

            w_pT1 = pw.tile([PDIM, DIM], f16)
            nc.sync.dma_start(w_pT1[:], projT1_d[:])
            w_pT2 = pw.tile([PDIM, DIM], f16)
            nc.sync.dma_start(w_pT2[:], projT2_d[:])
            w_temp = pw.tile([PDIM, 1], f32)
            nc.sync.dma_start(w_temp[:], tempb_d[:])
            w_bmask = pw.tile([PDIM, PDIM], f32)
            nc.sync.dma_start(w_bmask[:], bmask_d[:])
            w_bmask16 = pw.tile([PDIM, PDIM], f16)
            nc.sync.dma_start(w_bmask16[:], bmask16_d[:])
            w_maskJ = pw.tile([PDIM, CH], f16)
            nc.sync.dma_start(w_maskJ[:], maskJ_d[:])
            w_selB = pw.tile([PDIM, N_CORES], f32)
            nc.sync.dma_start(w_selB[:], selB_d[:])

            ident_f16 = pw.tile([128, 128], f16)
            masks.make_identity(nc, ident_f16[:])
            w_zero = pw.tile([128, 1], f32)
            nc.gpsimd.memset(w_zero[:], 0.0)
            w_negone = pw.tile([1, 1], f32)
            nc.gpsimd.memset(w_negone[:], -1.0)
            ones_row = pw.tile([1, 128], f32)
            nc.gpsimd.memset(ones_row[:], 1.0)

            # fp8 copy of v-group folded weights (taps 18..26)
            w_fold8v = pw.tile([PDIM, 9 * PDIM], f8)
            nc.vector.tensor_copy(w_fold8v[:], w_fold[:, 18 * PDIM:27 * PDIM])

            # ---- persistent big buffers ----
            x1cms = [pbig.tile([PDIM, BIR * WP], f16, name=f"x1cm{i}")
                     for i in range(2)]
            for t in x1cms:
                # zero only the pad columns (0, 257); per-block evictions
                # overwrite cols 1..256. Tiny DVE writes instead of a 3.9us
                # Pool memset that stalls the first block's evictions.
                t_r = t[:].rearrange("p (r w) -> p r w", w=WP)
                for pc in (0, WP - 1):
                    nc.vector.tensor_copy(
                        t_r[:, :, pc:pc + 1],
                        w_zero[:].unsqueeze(1).broadcast_to([PDIM, BIR, 1]))
            x8as = [pbig.tile([PDIM, BIR * WP], f8, name=f"x8a{i}")
                    for i in range(BLK)]
            x2buf = pbig.tile([PDIM, ROWS * W], f16)
            vtil = pbig.tile([PDIM, ROWS * W], f16)
            dwouts = [pbig.tile([PDIM, 2 * NPB], f16, name=f"dwout{i}")
                      for i in range(2)]
            sqcols = pbig.tile([PDIM, 64], f32)    # q: 0..31, k: 32..63
            gcols = pbig.tile([1, 32], f32)
            sgscr = pbig.tile([1, 512], f16)
            garm = pbig.tile([PDIM, PDIM], f32)
            arbuf = pbig.tile([PDIM, AGW], f32)
            garbuf = pbig.tile([PDIM, N_CORES * AGW], f32)

            gram_ps = pgram.tile([128, 128], f32, tag="gram")
            n_gram_mm = (NPB // 128) * BLK  # 128 total
            gram_i = 0

            # ================= PHASE 1 =================
            for blk in range(BLK):
                x1cm = x1cms[blk % 2]
                dwout = dwouts[blk % 2]
                x1_r = x1cm[:].rearrange("p (r w) -> p r w", w=WP)

                # --- load + convert + transpose x (4 quarters, 9 tiles) ---
                xqs = xq0s if blk == 0 else [load_quarter(blk, q)
                                             for q in range(NQ)]
                xq16s = []
                for q in range(NQ):
                    xq16 = pxq16.tile([128, QT * DIM], f16, tag="xq16")
                    if blk == 0:
                        # chunked conversion so transposes start early; DVE
                        # (Act is busy with act-table loads at startup)
                        for ck in range(3):
                            c0, c1 = ck * 3 * DIM, (ck + 1) * 3 * DIM
                            nc.vector.tensor_copy(xq16[:, c0:c1],
                                                  xqs[q][:, c0:c1])
                    else:
                        # Pool cast, chunked so transposes don't wait on the
                        # whole-quarter conversion latency
                        for ck in range(3):
                            c0, c1 = ck * 3 * DIM, (ck + 1) * 3 * DIM
                            nc.gpsimd.tensor_copy(xq16[:, c0:c1],
                                                  xqs[q][:, c0:c1])
                    xq16s.append(xq16)

                # px1 in [0:512] of a paired [128,1024] PSUM tile (bank-sized),
                # px2 (offset by 2 tiles) in [512:1024] of the same tile
                ptile = px2t = None
                for t in range(2 * BIR):  # 36 pixel-tiles of 128
                    src = xq16s[t // QT][:].rearrange("p (t c) -> p t c", c=DIM)
                    ti = t % QT
                    q1 = t % 4
                    if q1 == 0:
                        ptile = ptx.tile([128, 1024], f16, tag="ptx")
                    nc.tensor.transpose(ptile[:, q1 * 128:(q1 + 1) * 128],
                                        src[:, ti, 0:128], ident_f16[:])
                    if q1 == 3:
                        r = (t // 4) * 2
                        nc.vector.tensor_copy(
                            x1_r[:, r:r + 2, 1:1 + W],
                            ptile[:, 0:512].rearrange("p (r w) -> p r w", w=W))
                    if 2 <= t < 2 * BIR - 2:
                        q2 = (t - 2) % 4
                        if q2 == 0:
                            px2t = ptile
                        nc.tensor.transpose(
                            px2t[:, 512 + q2 * 128:512 + (q2 + 1) * 128],
                            src[:, ti, 128:256], ident_f16[:])
                        if q2 == 3:
                            gcol = blk * NPB + (t - 5) * 128
                            nc.vector.tensor_copy(x2buf[:, gcol:gcol + 512],
                                                  px2t[:, 512:1024])

                # fp8 copy of x1 for the v-conv (includes zero pads), chunked
                # so gate activations can interleave on Act
                for ck in range(3):
                    c0 = ck * 6 * WP
                    c1 = BIR * WP if ck == 2 else (ck + 1) * 6 * WP
                    nc.scalar.copy(x8as[blk][:, c0:c1], x1cm[:, c0:c1])

                # --- gate ---
                for chk in range(BR // 2):
                    g1p = pfold.tile([128, 512], f32, tag="pfold")
                    rhs1 = x1_r[:, 1 + chk * 2:3 + chk * 2, 1:1 + W]
                    nc.tensor.matmul(g1p[:], w_g1Ta[:], rhs1,
                                     start=True, stop=False)
                    rhs2 = x2buf[:, blk * NPB + chk * 512:blk * NPB + (chk + 1) * 512]
                    nc.tensor.matmul(g1p[:], w_g1Tb[:], rhs2,
                                     start=False, stop=True)
                    g1s = pg1.tile([128, 512], f16, tag="g1s")
                    nc.scalar.activation(g1s[:], g1p[:], AF.Relu, bias=w_g1b[:, 0:1])
                    row = blk * (BR // 2) + chk
                    nc.tensor.matmul(g1p[0:1, :], w_g2T[:], g1s[:],
                                     start=True, stop=True)
                    nc.scalar.activation(sgscr[:], g1p[0:1, :], AF.Sigmoid,
                                         bias=w_g2b[0:1, 0:1],
                                         accum_out=gcols[0:1, row:row + 1])

                # --- folded dwconv+qkv for q,k: taps-outer over 2-chunk pairs;
                # squares and qkT/gram interleaved per-hp so block-3's gram
                # (the collective's dependency) completes right after folds ---
                for g in range(2):
                    for hp in range(4):
                        pps = [pfold.tile([128, 512], f32, tag="pfold",
                                          name=f"pp{s}") for s in range(2)]
                        for tidx in range(9):
                            dy, dx = TAPS[tidx]
                            lhsT = w_fold[:, (g * 9 + tidx) * 128:
                                          (g * 9 + tidx + 1) * 128]
                            for s in range(2):
                                r0 = (hp * 2 + s) * 2
                                rhs = x1_r[:, r0 + dy:r0 + dy + 2, dx:dx + W]
                                nc.tensor.matmul(pps[s][:], lhsT, rhs,
                                                 start=(tidx == 0),
                                                 stop=(tidx == 8))
                        for s in range(2):
                            col = g * NPB + (hp * 2 + s) * 512
                            if s % 2 == 0:
                                nc.vector.tensor_copy(dwout[:, col:col + 512],
                                                      pps[s][:])
                            else:
                                nc.scalar.copy(dwout[:, col:col + 512], pps[s][:])
                        for s in range(2):
                            chk = hp * 2 + s
                            col = blk * 8 + chk
                            if g == 0:
                                if s == 0:
                                    sqa = psq.tile([128, 512], f16, tag="sqa")
                                    nc.scalar.activation(
                                        sqa[:], dwout[:, chk * 512:(chk + 1) * 512],
                                        AF.Square, bias=w_zero[:, 0:1],
                                        accum_out=sqcols[:, col:col + 1])
                                else:
                                    sqa = psq.tile([128, 512], f16, tag="sqa")
                                    nc.vector.tensor_mul(
                                        sqa[:], dwout[:, chk * 512:(chk + 1) * 512],
                                        dwout[:, chk * 512:(chk + 1) * 512])
                                    nc.vector.reduce_sum(
                                        sqcols[:, col:col + 1], sqa[:],
                                        axis=mybir.AxisListType.X)
                            else:
                                sqb = psq.tile([128, 512], f16, tag="sqa")
                                nc.vector.tensor_mul(
                                    sqb[:],
                                    dwout[:, NPB + chk * 512:NPB + (chk + 1) * 512],
                                    dwout[:, NPB + chk * 512:NPB + (chk + 1) * 512])
                                nc.vector.reduce_sum(
                                    sqcols[:, 32 + col:33 + col], sqb[:],
                                    axis=mybir.AxisListType.X)
                        if g == 1:
                            # q,k transposes + gram for this hp's px tiles
                            tq = None
                            for t2 in range(4 * hp, 4 * hp + 4):
                                if t2 % 2 == 0:
                                    tq = ptq.tile([128, 1024], f16, tag="ptq")
                                tb = (t2 % 2) * 512
                                for k in range(2):
                                    tt = t2 * 2 + k
                                    nc.tensor.transpose(
                                        tq[:, tb + k * 128:tb + (k + 1) * 128],
                                        dwout[:, tt * 128:(tt + 1) * 128],
                                        ident_f16[:])
                                    nc.tensor.transpose(
                                        tq[:, tb + 256 + k * 128:
                                           tb + 256 + (k + 1) * 128],
                                        dwout[:, NPB + tt * 128:
                                              NPB + (tt + 1) * 128],
                                        ident_f16[:])
                                ev = pev.tile([128, 512], f16, tag="ev")
                                nc.vector.tensor_copy(ev[:], tq[:, tb:tb + 512])
                                for k in range(2):
                                    nc.tensor.matmul(
                                        gram_ps[:], ev[:, k * 128:(k + 1) * 128],
                                        ev[:, 256 + k * 128:256 + (k + 1) * 128],
                                        start=(gram_i == 0),
                                        stop=(gram_i == n_gram_mm - 1))
                                    gram_i += 1

            # ================= AllGather staging =================
            # compact gram to per-head diag blocks [128,16]
            nc.vector.tensor_mul(garm[:], gram_ps[:], w_bmask[:])
            nc.vector.reduce_sum(
                arbuf[:, 0:16], garm[:].rearrange("p (d j) -> p j d", j=16),
                axis=mybir.AxisListType.X)
            nc.vector.reduce_sum(arbuf[:, 16:17], sqcols[:, 0:32],
                                 axis=mybir.AxisListType.X)
            nc.vector.reduce_sum(arbuf[:, 17:18], sqcols[:, 32:64],
                                 axis=mybir.AxisListType.X)
            nc.vector.tensor_copy(arbuf[:, 18:19], w_zero[:])
            nc.vector.reduce_sum(arbuf[0:1, 18:19], gcols[0:1, :],
                                 axis=mybir.AxisListType.X)
            inb = pdram.tile([PDIM, AGW], f32, tag="inb")
            # AllGather concatenates raveled per-rank buffers: [s, p, j]
            outag = pdram.tile([N_CORES * PDIM, AGW], f32, tag="outag",
                               addr_space="Shared")
            nc.sync.dma_start(inb[:], arbuf[:])
            nc.gpsimd.collective_compute(
                "AllGather", mybir.AluOpType.bypass,
                replica_groups=[list(range(N_CORES))],
                ins=[inb[:].opt()], outs=[outag[:].opt()])
            # gpsimd queue: a DMA holds its issuing queue's SEQ while waiting
            # for the collective; Pool is idle here, sync is not
            nc.gpsimd.dma_start(
                garbuf[:].rearrange("p (s j) -> p s j", j=AGW),
                outag[:].rearrange("(s p) j -> p s j", p=PDIM))

            # ================= v~ folds (hide the collective) =================
            # blocks 0-1 on the phase-1 pool (overlapping its drain barrier),
            # blocks 2-3 + attn + phase2 on a deep 7-bank pool
            pp2 = None
            wf8_r = w_fold8v[:].rearrange("p (t m) -> p t m", m=128)
            for blk in range(BLK):
                if blk == 2:
                    ph1.close()
                    pp2 = ctx.enter_context(
                        tc.tile_pool(name="pp2", bufs=7, space="PSUM"))
                vpool, vtag = (pfold, "pfold") if blk < 2 else (pp2, "vps")
                x8_r = x8as[blk][:].rearrange("p (r w) -> p r w", w=WP)
                for hp in range(4):
                    pps = [vpool.tile([128, 512], f32, tag=vtag,
                                      name=f"vp{s}") for s in range(2)]
                    # plain fp8 lone tap first: starts (zeroes) the full tile
                    dyL, dxL = TAPS[VLONE]
                    lhsTL = wf8_r[:, VLONE, :]
                    for s in range(2):
                        r0 = (hp * 2 + s) * 2
                        rhs = x8_r[:, r0 + dyL:r0 + dyL + 2, dxL:dxL + W]
                        nc.tensor.matmul(pps[s][:], lhsTL, rhs,
                                         start=True, stop=False)
                    for pi, (ta, tb) in enumerate(VPAIRS):
                        dyA, dxA = TAPS[ta]
                        dyB, dxB = TAPS[tb]
                        delta = (dyB - dyA) * WP + (dxB - dxA)
                        lhsT = wf8_r[:, ta:ta + 1, :].copy()
                        lhsT.ap[1] = [(tb - ta) * 128, 2]  # [128, 2, 128]
                        for s in range(2):
                            r0 = (hp * 2 + s) * 2
                            for r in range(2):
                                w0 = x8_r[:, r0 + dyA + r:r0 + dyA + r + 1,
                                          dxA:dxA + W]
                                wpair = w0.copy()
                                wpair.ap[1] = [delta, 2]
                                nc.tensor.matmul(
                                    pps[s][:, r * 256:(r + 1) * 256],
                                    lhsT, wpair, start=False,
                                    stop=(pi == 3 and r == 1), perf_mode=DR,
                                    skip_group_check=True)
                    for s in range(2):
                        col = blk * NPB + (hp * 2 + s) * 512
                        if s % 2 == 0:
                            nc.vector.tensor_copy(vtil[:, col:col + 512],
                                                  pps[s][:])
                        else:
                            nc.scalar.copy(vtil[:, col:col + 512], pps[s][:])

            # pre-issue the x2 half of phase-2 segment 0: it doesn't need the
            # attn result and bridges part of the collective latency
            pre2 = {}
            for oh in range(2):
                for c in range(2):
                    pt = pp2.tile([128, 512], f32, tag="vps", name="pre")
                    lx = w_pT2[:, oh * 128:(oh + 1) * 128]
                    gp = c * 512
                    nc.tensor.matmul(pt[:], lx, x2buf[:, gp:gp + 512],
                                     start=True, stop=False)
                    pre2[(oh, c)] = pt

            # ================= attn (tiny per-head CxC) =================
            # batch-local shard sums: agg[p, j] = sum_s sel[s] * shard_s[p, j]
            gar_r = garbuf[:].rearrange("p (s j) -> p s j", j=AGW)
            agsel = psm.tile([PDIM, N_CORES * AGW], f32)
            ag_r = agsel[:].rearrange("p (s j) -> p s j", j=AGW)
            nc.vector.tensor_mul(
                ag_r[:, :, :], gar_r[:, :, :],
                w_selB[:].unsqueeze(2).broadcast_to([PDIM, N_CORES, AGW]))
            agg = psm.tile([PDIM, AGW], f32)
            nc.vector.reduce_sum(
                agg[:], agsel[:].rearrange("p (s j) -> p j s", j=AGW),
                axis=mybir.AxisListType.X)
            # gate global sum over all 8 shards
            gsum = psm.tile([1, 1], f32)
            nc.vector.reduce_sum(
                gsum[:], gar_r[0:1, :, 18:19].rearrange("p s j -> p (j s)"),
                axis=mybir.AxisListType.X)
            # threshold = 16*mean(g) - 1, broadcast to partitions
            thr = psm.tile([1, 1], f32)
            nc.scalar.activation(thr[:], gsum[0:1, 0:1], AF.Identity,
                                 scale=float(CH) / float(B * H * W),
                                 bias=w_negone[0:1, 0:1])
            thrB_ps = pp2.tile([128, 512], f32, tag="vps")
            nc.tensor.matmul(thrB_ps[:, 0:1], ones_row[:], thr[:],
                             start=True, stop=True)
            thr_b = psm.tile([128, 1], f32)
            nc.scalar.copy(thr_b[:], thrB_ps[:, 0:1])
            # norms: nq = temp/max(sqrt(sq_q),eps), nk = 1/max(sqrt(sq_k),eps)
            nq = psm.tile([128, 1], f32)
            nc.scalar.activation(nq[:], agg[:, 16:17], AF.Sqrt,
                                 bias=w_zero[:, 0:1])
            nc.vector.tensor_scalar_max(nq[:], nq[:], 1e-12)
            nc.vector.reciprocal(nq[:], nq[:])
            nc.vector.tensor_mul(nq[:], nq[:], w_temp[:])
            nk = psm.tile([128, 1], f32)
            nc.scalar.activation(nk[:], agg[:, 17:18], AF.Sqrt,
                                 bias=w_zero[:, 0:1])
            nc.vector.tensor_scalar_max(nk[:], nk[:], 1e-12)
            nc.vector.reciprocal(nk[:], nk[:])
            # nkB16[c, j] = nk[head(c)*16 + j] via bmask matmul trick
            nkS = psm.tile([128, CH], f16)
            nc.vector.tensor_scalar_mul(nkS[:], w_maskJ[:], nk[:, 0:1])
            nkB_ps = pp2.tile([128, 512], f32, tag="vps")
            nc.tensor.matmul(nkB_ps[:, 0:CH], w_bmask16[:], nkS[:],
                             start=True, stop=True)
            at16 = psm.tile([128, CH], f32)
            nc.vector.scalar_tensor_tensor(at16[:], agg[:, 0:16], nq[:, 0:1],
                                           nkB_ps[:, 0:CH],
                                           op0=ALU.mult, op1=ALU.mult)

            # ranks: rk[p,j] = #{j' : at16[p,j'] > at16[p,j]} via one broadcast
            # compare [128,16,16] + reduce over j'
            cmp = psm.tile([128, CH * CH], f32)
            nc.vector.tensor_tensor(
                cmp[:].rearrange("p (a b) -> p a b", b=CH),
                at16[:].unsqueeze(2).broadcast_to([128, CH, CH]),
                at16[:].unsqueeze(1).broadcast_to([128, CH, CH]),
                op=ALU.is_lt)
            rk = psm.tile([128, CH], f32)
            nc.vector.reduce_sum(rk[:],
                                 cmp[:].rearrange("p (a b) -> p a b", b=CH),
                                 axis=mybir.AxisListType.X)
            mask = psm.tile([128, CH], f32)
            nc.vector.tensor_scalar(mask[:], rk[:], thr_b[:, 0:1], None,
                                    op0=ALU.is_lt)
            # masked softmax (global row max is always kept, k >= 1)
            mx = psm.tile([128, 1], f32)
            nc.vector.reduce_max(mx[:], at16[:], axis=mybir.AxisListType.X)
            mxn = psm.tile([128, 1], f32)
            nc.vector.tensor_scalar_mul(mxn[:], mx[:], -1.0)
            e16 = psm.tile([128, CH], f32)
            nc.scalar.activation(e16[:], at16[:], AF.Exp, bias=mxn[:, 0:1])
            nc.vector.tensor_mul(e16[:], e16[:], mask[:])
            ssum = psm.tile([128, 1], f32)
            nc.vector.reduce_sum(ssum[:], e16[:], axis=mybir.AxisListType.X)
            nc.vector.reciprocal(ssum[:], ssum[:])
            nc.vector.tensor_scalar_mul(e16[:], e16[:], ssum[:, 0:1])

            # P1eff = blockdiag(attn) @ projT1s  (asum folded host-side)
            bd = psm.tile([128, 128], f16)
            nc.vector.tensor_mul(
                bd[:].rearrange("p (d j) -> p d j", j=16),
                e16[:].unsqueeze(1).broadcast_to([128, HEADS, 16]),
                w_bmask16[:].rearrange("p (d j) -> p d j", j=16))
            p1p = pp2.tile([128, 512], f32, tag="vps")
            nc.tensor.matmul(p1p[:, 0:DIM], bd[:], w_pT1[:],
                             start=True, stop=True)
            p1eff = psm.tile([PDIM, DIM], f16)
            nc.scalar.copy(p1eff[:], p1p[:, 0:DIM])

            if DEBUG:
                nc.sync.dma_start(dbg["d_x2buf"][:], x2buf[:])
                nc.sync.dma_start(dbg["d_vtil"][:], vtil[:])
                nc.sync.dma_start(dbg["d_dwout3"][:], dwouts[1][:])
                nc.sync.dma_start(dbg["d_garbuf"][:], garbuf[:])
                nc.sync.dma_start(dbg["d_arbuf"][:], arbuf[:])
                nc.sync.dma_start(dbg["d_agg"][:], agg[:])
                nc.sync.dma_start(dbg["d_at16"][:], at16[:])
                nc.sync.dma_start(dbg["d_rk"][:], rk[:])
                nc.sync.dma_start(dbg["d_e16"][:], e16[:])
                nc.sync.dma_start(dbg["d_p1eff"][:], p1eff[:])
                nc.sync.dma_start(dbg["d_x8a0"][:], x8as[0][:])

            # ================= PHASE 2: transposed proj + output ============
            # outT[o, px] = P1eff[:,o]^T @ v~ + projT2[:,o]^T @ x2
            for seg in range(ROWS * W // 1024):  # 16 segments of 1024 px
                for oh in range(2):
                    if seg == 0:
                        pps = [pre2[(oh, c)] for c in range(2)]
                    else:
                        pps = [pp2.tile([128, 512], f32, tag="vps",
                                        name=f"o{c}") for c in range(2)]
                        # x2 half first: independent of the attn result
                        lx = w_pT2[:, oh * 128:(oh + 1) * 128]
                        for c in range(2):
                            gp = seg * 1024 + c * 512
                            nc.tensor.matmul(pps[c][:], lx,
                                             x2buf[:, gp:gp + 512],
                                             start=True, stop=False)
                    lv = p1eff[:, oh * 128:(oh + 1) * 128]
                    for c in range(2):
                        gp = seg * 1024 + c * 512
                        nc.tensor.matmul(pps[c][:], lv, vtil[:, gp:gp + 512],
                                         start=False, stop=True)
                    ot = pout.tile([128, 1024], bf16, tag="ot")
                    nc.vector.tensor_copy(ot[:, 0:512], pps[0][:])
                    nc.scalar.copy(ot[:, 512:1024], pps[1][:])
                    dst = ys[oh * 128:(oh + 1) * 128,
                             seg * 1024:(seg + 1) * 1024]
                    if (seg * 2 + oh) % 2 == 1:
                        nc.sync.dma_start(dst, ot[:])
                    else:
                        nc.gpsimd.dma_start(dst, ot[:])

    nc.finalize()
    return nc


_CACHED = {}


def _get_results(in_maps):
    from concourse.bass_utils import run_bass_kernel_spmd
    if "nc" not in _CACHED:
        _CACHED["nc"] = _build_program()
    nc = _CACHED["nc"]
    return run_bass_kernel_spmd(nc, in_maps, list(range(N_CORES)))


def _prep_inputs(x, qkv_w, dw_w, proj_w, g1_w, g1_b, g2_w, g2_b,
                 temperature, attn1, attn2, attn3, attn4):
    x = np.asarray(x, np.float32)
    wT = np.ascontiguousarray(np.asarray(qkv_w, np.float32).T)         # [128, 384]
    dwf = np.asarray(dw_w, np.float32).reshape(3 * PDIM, 9)            # [384, 9]
    # wfold[c, g, t, o] = wT[c, g*128+o] * dwf[g*128+o, t]
    wfold = (wT.reshape(PDIM, 3, 1, PDIM)
             * dwf.reshape(3, PDIM, 9).transpose(0, 2, 1)[None])       # [128,3,9,128]
    wfold = np.ascontiguousarray(wfold.reshape(PDIM, 27 * PDIM)).astype(np.float16)
    g1wT = np.asarray(g1_w, np.float32).T                              # [256, 128]
    g1wTa = np.ascontiguousarray(g1wT[0:PDIM]).astype(np.float16)
    g1wTb = np.ascontiguousarray(g1wT[PDIM:2 * PDIM]).astype(np.float16)
    g1b = np.asarray(g1_b, np.float32).reshape(PDIM, 1)
    g2T = np.asarray(g2_w, np.float32).reshape(PDIM, 1).astype(np.float16)
    g2b = np.asarray(g2_b, np.float32).reshape(1, 1)
    projT = np.ascontiguousarray(np.asarray(proj_w, np.float32).T)     # [256, 256]
    tempb = np.repeat(np.asarray(temperature, np.float32).reshape(HEADS), CH)
    tempb = np.ascontiguousarray(tempb.reshape(PDIM, 1))
    bmask = np.zeros((PDIM, PDIM), np.float32)
    for h in range(HEADS):
        bmask[h * CH:(h + 1) * CH, h * CH:(h + 1) * CH] = 1.0
    bmask16 = bmask.astype(np.float16)
    maskJ = np.zeros((PDIM, CH), np.float16)
    for d in range(PDIM):
        maskJ[d, d % CH] = 1.0
    asum = float(sum(np.asarray(a, np.float32).reshape(-1)[0]
                     for a in (attn1, attn2, attn3, attn4)))
    projT1 = np.ascontiguousarray(projT[0:PDIM] * asum).astype(np.float16)
    projT2 = np.ascontiguousarray(projT[PDIM:2 * PDIM]).astype(np.float16)

    in_maps = []
    for cid in range(N_CORES):
        b = cid // 4
        r0 = (cid % 4) * ROWS
        xsh = np.zeros((HALO_ROWS, W, DIM), np.float32)
        lo, hi = r0 - 1, r0 + ROWS + 1
        slo, shi = max(lo, 0), min(hi, H)
        xsh[slo - lo:shi - lo] = x[b, slo:shi]
        selB = np.zeros((PDIM, N_CORES), np.float32)
        selB[:, 4 * b:4 * b + 4] = 1.0
        in_maps.append(dict(
            xs=np.ascontiguousarray(xsh.reshape(HALO_ROWS * W, DIM)),
            wfold=wfold, g1wTa=g1wTa, g1wTb=g1wTb, g1b=g1b,
            g2T=g2T, g2b=g2b,
            projT1=projT1, projT2=projT2, tempb=tempb,
            bmask=bmask, bmask16=bmask16, maskJ=maskJ, selB=selB,
        ))
    return in_maps


def kernel(x, qkv_w, dw_w, proj_w, g1_w, g1_b, g2_w, g2_b,
           temperature, attn1, attn2, attn3, attn4):
    in_maps = _prep_inputs(x, qkv_w, dw_w, proj_w, g1_w, g1_b, g2_w, g2_b,
                           temperature, attn1, attn2, attn3, attn4)
    res = _get_results(in_maps)
    out = np.zeros((B, H, W, DIM), np.float32)
    for cid in range(N_CORES):
        b = cid // 4
        r0 = (cid % 4) * ROWS
        yt = np.asarray(res.results[cid]["ys"], np.float32)  # [DIM, ROWS*W]
        out[b, r0:r0 + ROWS] = yt.reshape(DIM, ROWS, W).transpose(1, 2, 0)
    return out
